# revision 41
# baseline (speedup 1.0000x reference)
"""LeViT-style attention (B=32, N=784, DIM=512, H=8, KD=32, VD=128) on 8 trn2 cores.

Strategy: pure data-parallel over batch (4 batches/core, no collectives).
Host folds BN into weights/biases, folds the softmax scale into Wq, and
precomputes EB = exp(rel-pos-bias) (multiplicative form). Per core:
  stage1: qkT = Wqk.T @ xT   (bf16, head-grouped channel layout; bias via
          ACT Identity+bias), v = xT.T @ Wv (token-major, bf16, NO bias:
          since sum_m P[n,m] == 1 the v-bias is folded into the hardswish
          relu bias as (3 + bv[d]) per-partition). The 16-row tail tokens
          of all 4 batches are packed into one 128-wide stationary group
          (32-partition aligned, zero-padded).
  stage2: software-pipelined over (head-pair, batch): softmax of batch b
          overlaps AV/hardswish/projection of batch b-1 (emitted as filler
          groups >=2 slots after the chains they consume; persistent pt
          buffers + emission-order/region WAR semaphores give the sync;
          bias-table chunks stream just-in-time per slot). Per tile:
          S = q.kT (K=32 matmuls, packed row groups, 3-way rotating PSUM
          tiles so the S->exp WAR never stalls); exp on ScalarE (no
          accumulator read); P~ = exp(S)*EB on the Pool engine (TT, SBUF);
          softmax denominator via DVE tensor_scalar accum in 4x mode;
          P normalized on DVE (4x); P^T via a SINGLE padded [nr, 896] bf16
          SBUF->SBUF DMA-transpose per (tile, head); o^T = v.T @ P^T;
          hardswish as h=(t-3)*min(t,6)/6 with t=relu(o+bv+3) in f32
          (relu on DVE, min on GpSimd); output projection (bf16, 1/6
          folded into Wp); out written bf16 and upcast on host.
"""
import numpy as np
import ml_dtypes

import concourse.bass as bass
import concourse.mybir as mybir
import concourse.tile as tile
from concourse import bacc
from concourse.bass_utils import run_bass_kernel_spmd

F32 = mybir.dt.float32
BF16 = mybir.dt.bfloat16
AF = mybir.ActivationFunctionType
OP = mybir.AluOpType

B, N, DIM = 32, 784, 512
H, KD, VD, RES = 8, 32, 128, 28
NCORES = 8
BL = B // NCORES          # batches per core = 4
NL = BL * N               # 3136 tokens per core
NT = 7                    # n-tiles per batch (6x128 + 16)
MC = 7                    # m-chunks per batch
EPS = 1e-5


def _rows(i):
    return 128 if i < 6 else 16


def build_nc():
    nc = bacc.Bacc(None, target_bir_lowering=False, debug=False)

    xT = nc.dram_tensor("xT", [DIM, NL], BF16, kind="ExternalInput")
    wqk = nc.dram_tensor("wqk", [DIM, 512], BF16, kind="ExternalInput")
    bqk = nc.dram_tensor("bqk", [128, 4], F32, kind="ExternalInput")
    wv = nc.dram_tensor("wv", [DIM, 1024], BF16, kind="ExternalInput")
    bv3 = nc.dram_tensor("bv3", [128, 8], F32, kind="ExternalInput")
    wp = nc.dram_tensor("wp", [1024, 512], BF16, kind="ExternalInput")
    bp = nc.dram_tensor("bp", [128, 512], F32, kind="ExternalInput")
    eb = nc.dram_tensor("ebias", [H, 896, 784], BF16, kind="ExternalInput")
    out = nc.dram_tensor("out", [NL, 512], BF16, kind="ExternalOutput")

    with tile.TileContext(nc) as tc:
        with (
            tc.tile_pool(name="persist", bufs=1) as pp,
            tc.tile_pool(name="att", bufs=2) as ap_,
            tc.tile_pool(name="ebp", bufs=1) as ebp,
            tc.tile_pool(name="mmps", bufs=2, space="PSUM") as mm_pool,
        ):
            # persistent SBUF tensors
            qkT = pp.tile([128, 4, NL], BF16)      # q/k channels, head-grouped
            v_sb = pp.tile([128, BL, MC, 1024], BF16)
            wp_sb = pp.tile([128, 8, 512], BF16)
            bp_sb = pp.tile([128, 512], F32)
            bv3_sb = pp.tile([128, 8], F32)

            # ---------------- stage 1: projections ----------------
            with tc.tile_pool(name="s1", bufs=1) as s1:
                xT_sb = s1.tile([128, 4, NL], BF16)
                wqk_sb = s1.tile([128, 4, 512], BF16)
                wv_sb = s1.tile([128, 4, 1024], BF16)
                bqk_sb = s1.tile([128, 4], F32)
                xT_r = xT[:].rearrange("(cc p) n -> p cc n", p=128)
                wqk_r = wqk[:].rearrange("(cc p) o -> p cc o", p=128)
                wv_r = wv[:].rearrange("(cc p) o -> p cc o", p=128)
                # merged 3D stripe DMAs: each transfer covers all 4 channel
                # chunks, staying above the per-transfer DMA floor
                nc.sync.dma_start(wqk_sb[:], wqk_r[:])
                nc.sync.dma_start(xT_sb[:, :, 0:448], xT_r[:, :, 0:448])
                nc.sync.dma_start(bqk_sb[:], bqk[:])
                nc.sync.dma_start(bv3_sb[:], bv3[:])
                for ntc in range(1, NL // 448):
                    nc.sync.dma_start(
                        xT_sb[:, :, ntc * 448:(ntc + 1) * 448],
                        xT_r[:, :, ntc * 448:(ntc + 1) * 448])
                for cc in range(4):
                    nc.sync.dma_start(wv_sb[:, cc], wv_r[:, cc])

                # qkT[o, n] accumulation over 4 c-chunks; 448-wide n stripes
                for ntc in range(NL // 448):
                    for oc in range(4):
                        ps = mm_pool.tile([128, 512], F32, tag="mm")
                        for cc in range(4):
                            nc.tensor.matmul(
                                ps[:, :448],
                                wqk_sb[:, cc, oc * 128:(oc + 1) * 128],
                                xT_sb[:, cc, ntc * 448:(ntc + 1) * 448],
                                start=(cc == 0), stop=(cc == 3),
                            )
                        nc.scalar.activation(
                            qkT[:, oc, ntc * 448:(ntc + 1) * 448], ps[:, :448],
                            AF.Identity, bias=bqk_sb[:, oc:oc + 1],
                        )

                # v[n, vd] token-major (raw: bv folded into hardswish bias)
                for b4 in range(BL):
                    for mc in range(6):
                        col0 = b4 * N + mc * 128
                        for vh in range(2):
                            ps = mm_pool.tile([128, 512], F32, tag="mm")
                            for cc in range(4):
                                nc.tensor.matmul(
                                    ps[:],
                                    xT_sb[:, cc, col0:col0 + 128],
                                    wv_sb[:, cc, vh * 512:(vh + 1) * 512],
                                    start=(cc == 0), stop=(cc == 3),
                                )
                            nc.scalar.activation(
                                v_sb[:, b4, mc, vh * 512:(vh + 1) * 512],
                                ps[:], AF.Copy)
                # all 4 batches' 16-row tail tokens in one matmul group:
                # slivers staged 32-partition-aligned with zero padding
                xsl = s1.tile([128, 4, 128], BF16)
                vst = s1.tile([128, 2, 512], BF16)
                nc.vector.memset(xsl[:], 0.0)
                for cc in range(4):
                    nc.vector.tensor_copy(
                        xsl[:, cc].rearrange("p (b j) -> p b j", b=BL)[:, :, :16],
                        xT_sb[:, cc].rearrange(
                            "p (b n) -> p b n", b=BL)[:, :, 768:784])
                for vh in range(2):
                    ps = mm_pool.tile([128, 512], F32, tag="mm")
                    for cc in range(4):
                        nc.tensor.matmul(
                            ps[:],
                            xsl[:, cc],
                            wv_sb[:, cc, vh * 512:(vh + 1) * 512],
                            start=(cc == 0), stop=(cc == 3),
                        )
                    nc.scalar.activation(vst[:, vh], ps[:], AF.Copy)
                for b4 in range(BL):
                    nc.sync.dma_start(
                        v_sb[:16, b4, 6, :],
                        vst[32 * b4:32 * b4 + 16])

            # ---------------- stage 2: attention ----------------
            from functools import partial
            with (
                tc.tile_pool(name="p2", bufs=1) as pp2,
                tc.tile_pool(name="sps", bufs=1, space="PSUM") as s_pool,
            ):
                hT = pp2.tile([128, BL, H, N], BF16)
                # persistent P^T buffers, one per head slot; pipeline sync
                # comes from emission order + region-level WAR/RAW deps
                pt_bufs = [pp2.tile([128, MC, N], BF16, name=f"ptb{i}")
                           for i in range(2)]
                # persistent eb tables, DMA'd chunkwise so the next head
                # pair's table streams in behind the current reads
                ebt = [pp2.tile([128, MC, 784], BF16, name=f"ebt{i}")
                       for i in range(2)]
                # persistent round-robin es buffers (4 per head slot) with a
                # zeroed transpose pad
                es_bufs = [pp2.tile([128, 896], BF16, name=f"esb{j}")
                           for j in range(8)]
                for t in es_bufs:
                    nc.vector.memset(t[:, 784:], 0.0)
                es_cnt = [0, 0]
                s_rot = [0]
                hsl = pp2.tile([128, 8, 128], BF16, name="hsl")
                nc.vector.memset(hsl[:], 0.0)

                def eb_chunk(hp_t, i, mc):
                    ebr = eb[2 * hp_t + i].rearrange(
                        "(mc p) n -> p mc n", p=128)
                    nc.sync.dma_start(ebt[i][:, mc], ebr[:, mc])

                def softmax_tile(hp, b4, nt, defer_tp=None):
                    nr = _rows(nt)
                    c0 = b4 * N + nt * 128
                    nsl = slice(nt * 128, nt * 128 + nr)
                    # 3-way rotating S tiles shared by both heads (global
                    # stream counter): the WAR partner of each S matmul is
                    # the exp 3 stream-slots back, so the S->exp ping-pong
                    # never stalls either chain, including at batch bounds
                    Ss = []
                    for i in range(2):
                        j = s_rot[0] % 3
                        s_rot[0] += 1
                        Ss.append(s_pool.tile([128, 2, 512], F32,
                                              tag=f"sp{j}", name=f"sp{j}"))
                    for i in range(2):
                        h = 2 * hp + i
                        ccq, cck = (0, 1) if h < 4 else (2, 3)
                        pq = 32 * (h % 4)
                        for half in range(2):
                            m0 = b4 * N + half * 392
                            nc.tensor.matmul(
                                Ss[i][:nr, half, :392],
                                qkT[pq:pq + 32, ccq, c0:c0 + nr],
                                qkT[pq:pq + 32, cck, m0:m0 + 392],
                                start=True, stop=True,
                                tile_position=(pq, 0))
                    for i in range(2):
                        es = es_bufs[4 * i + es_cnt[i] % 4]
                        es_cnt[i] += 1
                        den = ap_.tile([128, 1], F32, tag=f"den{i}")
                        nc.scalar.activation(
                            es[:nr, :784].rearrange("p (a b) -> p a b", a=2),
                            Ss[i][:nr, :, :392], AF.Exp)
                        nc.gpsimd.tensor_tensor(
                            es[:nr, :784], es[:nr, :784],
                            ebt[i][:nr, nt, :], OP.mult)
                        nc.vector.tensor_scalar(
                            es[:nr, :784], es[:nr, :784], 1.0, 0.0,
                            OP.mult, OP.add, accum_out=den[:nr])
                        rd = ap_.tile([128, 1], F32, tag=f"rd{i}")
                        nc.vector.reciprocal(rd[:nr], den[:nr])
                        nc.vector.tensor_scalar_mul(
                            es[:nr, :784], es[:nr, :784], rd[:nr, 0:1])
                        tp = partial(nc.sync.dma_start_transpose,
                                     pt_bufs[i][:, :, nsl], es[:nr, :896])
                        if defer_tp is None:
                            tp()
                        else:
                            defer_tp.append(tp)

                def av_group(hp, b4, k):
                    # one (head, col-half) AV + hardswish group of batch b4
                    i = k // 2
                    hs, hw_ = ((0, 448), (448, 336))[k % 2]
                    h = 2 * hp + i
                    ops = mm_pool.tile([128, 512], F32, tag="mm")
                    for mc in range(MC):
                        mr = _rows(mc)
                        nc.tensor.matmul(
                            ops[:, :hw_],
                            v_sb[:mr, b4, mc, h * 128:(h + 1) * 128],
                            pt_bufs[i][:mr, mc, hs:hs + hw_],
                            start=(mc == 0), stop=(mc == 6))
                    tt = ap_.tile([128, 448], F32, tag="hsw")
                    nc.vector.tensor_scalar(
                        tt[:, :hw_], ops[:, :hw_],
                        bv3_sb[:, h:h + 1], 0.0, OP.add, OP.max)
                    mm_ = ap_.tile([128, 448], F32, tag="mmin")
                    nc.gpsimd.tensor_scalar_min(
                        mm_[:, :hw_], tt[:, :hw_], 6.0)
                    nc.vector.scalar_tensor_tensor(
                        hT[:, b4, h, hs:hs + hw_],
                        tt[:, :hw_], -3.0, mm_[:, :hw_],
                        OP.add, OP.mult)

                def proj_group(b4, nt):
                    c0 = b4 * N + nt * 128
                    ps = mm_pool.tile([128, 512], F32, tag="mm")
                    for hh in range(8):
                        nc.tensor.matmul(
                            ps[:],
                            hT[:, b4, hh, nt * 128:nt * 128 + 128],
                            wp_sb[:, hh, :],
                            start=(hh == 0), stop=(hh == 7))
                    ob = ap_.tile([128, 512], BF16, tag="ob")
                    nc.vector.tensor_tensor(ob[:], ps[:], bp_sb[:], OP.add)
                    nc.sync.dma_start(out[c0:c0 + 128, :], ob[:])

                def proj_sliver():
                    # 16-row tail tokens of all 4 batches, 32-aligned packing
                    for hh in range(8):
                        nc.vector.tensor_copy(
                            hsl[:, hh].rearrange(
                                "p (b j) -> p b j", b=BL)[:, :, :16],
                            hT[:, :, hh, 768:784])
                    ps = mm_pool.tile([128, 512], F32, tag="mm")
                    for hh in range(8):
                        nc.tensor.matmul(
                            ps[:], hsl[:, hh], wp_sb[:, hh, :],
                            start=(hh == 0), stop=(hh == 7))
                    obs = ap_.tile([128, 512], BF16, tag="ob")
                    nc.vector.tensor_tensor(obs[:], ps[:], bp_sb[:], OP.add)
                    for b4 in range(BL):
                        nc.sync.dma_start(
                            out[b4 * N + 768:b4 * N + 784, :],
                            obs[32 * b4:32 * b4 + 16])

                # schedule: slot s of batch b4 runs softmax_tile(nt=s) then
                # filler groups, placed >=2 slots after the softmax chain
                # they consume: hs=448 AV groups of the PREVIOUS batch in
                # slots 2-3 (chain of its nt6 ends ~2 slots into this batch;
                # pt cols 448:784 are only overwritten from nt=3 on, emitted
                # after); hs=0 AV groups of the CURRENT batch in slots 5-6
                # (their pt cols 0:448 complete at nt=3); projections of the
                # previous batch follow its hs=448 hardswish in slots 4-6.
                nc.sync.dma_start(
                    wp_sb[:], wp[:].rearrange("(hh p) o -> p hh o", p=128))
                nc.sync.dma_start(bp_sb[:], bp[:])
                prev = None
                for hp in range(4):
                    for b4 in range(BL):
                        slotfill = [[] for _ in range(NT)]
                        if prev is not None:
                            php, pb4 = prev
                            # both hs=448 groups must precede this batch's
                            # nt3 transpose (it overwrites pt cols 448:512);
                            # that transpose is deferred to slot 4, so slot 3
                            # is the last legal (and least stalled) position
                            slotfill[3].append(partial(av_group, php, pb4, 1))
                            slotfill[3].append(partial(av_group, php, pb4, 3))
                            if php == 3:
                                pj = [partial(proj_group, pb4, n_)
                                      for n_ in range(6)]
                                slotfill[4] += pj[0:2]
                                slotfill[5] += pj[2:4]
                                slotfill[6] += pj[4:6]
                        slotfill[5].append(partial(av_group, hp, b4, 0))
                        slotfill[6].append(partial(av_group, hp, b4, 2))
                        for nt in range(NT):
                            # bias-table chunk loads, just-in-time and spread
                            # one chunk pair per slot: hp0 loads chunk nt in
                            # the slot that first reads it; later eras get
                            # chunks 0-3 prefetched at the previous batch 3
                            # and chunks 4-6 early in their own batch 0
                            if hp == 0 and b4 == 0:
                                for i in range(2):
                                    eb_chunk(0, i, nt)
                            elif b4 == 0 and hp > 0 and nt < 3:
                                for i in range(2):
                                    eb_chunk(hp, i, nt + 4)
                            if nt == 3:
                                # run the chain but hold nt3's transposes
                                # until after slot 3's hs=448 AV fillers
                                deferred = []
                                softmax_tile(hp, b4, nt, defer_tp=deferred)
                            else:
                                if nt == 4:
                                    for tp in deferred:
                                        tp()
                                softmax_tile(hp, b4, nt)
                            if b4 == BL - 1 and hp < 3 and nt < 4:
                                for i in range(2):
                                    eb_chunk(hp + 1, i, nt)
                            for f in slotfill[nt]:
                                f()
                        prev = (hp, b4)
                # drain: last batch's hs=448 AV groups + projection
                av_group(3, 3, 1)
                av_group(3, 3, 3)
                for n_ in range(6):
                    proj_group(3, n_)
                proj_sliver()

    nc.compile()
    return nc


_NC = None


def _prep_weights(qkv_w, qkv_g, qkv_b, qkv_m, qkv_v, ab, proj_w, proj_g,
                  proj_b, proj_m, proj_v, idxs):
    s = qkv_g / np.sqrt(qkv_v + EPS)
    W = qkv_w * s[:, None]                       # [1536, 512]
    bias = qkv_b - qkv_m * s                     # [1536]
    scale = KD ** -0.5
    # head-grouped reorder: chunk0=q0..3, chunk1=k0..3, chunk2=q4..7, chunk3=k4..7
    qk_rows, v_rows = [], []
    for h in range(H):
        base = h * (2 * KD + VD)
        qk_rows.append((np.arange(base, base + KD), True))
        qk_rows.append((np.arange(base + KD, base + 2 * KD), False))
        v_rows.append(np.arange(base + 2 * KD, base + 2 * KD + VD))
    order = []
    for grp in range(4):
        half = grp // 2
        is_q = (grp % 2 == 0)
        for hh in range(4 * half, 4 * half + 4):
            order.append((qk_rows[2 * hh][0] if is_q else qk_rows[2 * hh + 1][0], is_q))
    wqk = np.empty((512, 512), np.float32)
    bqk = np.empty(512, np.float32)
    o = 0
    for rows, is_q in order:
        f = scale if is_q else 1.0
        wqk[:, o:o + KD] = (W[rows] * f).T
        bqk[o:o + KD] = bias[rows] * f
        o += KD
    vr = np.concatenate(v_rows)
    wv = W[vr].T.copy()                          # [512, 1024]
    bv = bias[vr]                                # folded into hardswish bias

    sp = proj_g / np.sqrt(proj_v + EPS)
    # reference: out = h @ proj_w.T * sp + (proj_b - proj_m*sp); fold 1/6 of hswish
    wp = (proj_w * sp[:, None]).T.astype(np.float32) / 6.0   # [1024, 512]
    bp = proj_b - proj_m * sp

    btab = ab[:, idxs].astype(np.float32)                    # [H, 784, 784]
    eb_pad = np.zeros((H, 896, 784), np.float32)
    eb_pad[:, :784] = np.exp(btab)               # multiplicative bias

    return dict(
        wqk=wqk.astype(ml_dtypes.bfloat16),
        bqk=bqk.reshape(4, 128).T.copy(),
        wv=wv.astype(ml_dtypes.bfloat16),
        bv3=(3.0 + bv).reshape(8, 128).T.astype(np.float32).copy(),
        wp=wp.astype(ml_dtypes.bfloat16),
        bp=np.broadcast_to(bp, (128, 512)).astype(np.float32).copy(),
        ebias=eb_pad.astype(ml_dtypes.bfloat16),
    )


def kernel(x, qkv_w, qkv_g, qkv_b, qkv_m, qkv_v, ab,
           proj_w, proj_g, proj_b, proj_m, proj_v, idxs, _trace=False):
    global _NC
    x = np.asarray(x, np.float32)
    shared = _prep_weights(
        np.asarray(qkv_w, np.float32), np.asarray(qkv_g, np.float32),
        np.asarray(qkv_b, np.float32), np.asarray(qkv_m, np.float32),
        np.asarray(qkv_v, np.float32), np.asarray(ab, np.float32),
        np.asarray(proj_w, np.float32), np.asarray(proj_g, np.float32),
        np.asarray(proj_b, np.float32), np.asarray(proj_m, np.float32),
        np.asarray(proj_v, np.float32), np.asarray(idxs))

    if _NC is None:
        _NC = build_nc()
    nc = _NC

    in_maps = []
    for c in range(NCORES):
        xs = x[c * BL:(c + 1) * BL]                      # [4, 784, 512]
        xT = xs.transpose(2, 0, 1).reshape(DIM, NL).astype(ml_dtypes.bfloat16)
        m = dict(shared)
        m["xT"] = xT
        in_maps.append(m)

    res = run_bass_kernel_spmd(nc, in_maps, core_ids=list(range(NCORES)),
                               trace=_trace)
    outs = [res.results[c]["out"].astype(np.float32).reshape(BL, N, DIM)
            for c in range(NCORES)]
    full = np.concatenate(outs, axis=0)
    if _trace:
        return full, res.exec_time_ns
    return full


# revision 44
# speedup vs baseline: 1.0013x; 1.0013x over previous
"""LeViT-style attention (B=32, N=784, DIM=512, H=8, KD=32, VD=128) on 8 trn2 cores.

Strategy: pure data-parallel over batch (4 batches/core, no collectives).
Host folds BN into weights/biases, folds the softmax scale into Wq, and
precomputes EB = exp(rel-pos-bias) (multiplicative form). Per core:
  stage1: qkT = Wqk.T @ xT   (bf16, head-grouped channel layout; bias via
          ACT Identity+bias), v = xT.T @ Wv (token-major, bf16, NO bias:
          since sum_m P[n,m] == 1 the v-bias is folded into the hardswish
          relu bias as (3 + bv[d]) per-partition). The 16-row tail tokens
          of all 4 batches are packed into one 128-wide stationary group
          (32-partition aligned, zero-padded).
  stage2: software-pipelined over (head-pair, batch): softmax of batch b
          overlaps AV/hardswish/projection of batch b-1 (emitted as filler
          groups >=2 slots after the chains they consume; persistent pt
          buffers + emission-order/region WAR semaphores give the sync;
          bias-table chunks stream just-in-time per slot). Per tile:
          S = q.kT (K=32 matmuls, packed row groups, 3-way rotating PSUM
          tiles so the S->exp WAR never stalls); exp on ScalarE (no
          accumulator read); P~ = exp(S)*EB on the Pool engine (TT, SBUF);
          softmax denominator via DVE tensor_scalar accum in 4x mode;
          P normalized on DVE (4x); P^T via a SINGLE padded [nr, 896] bf16
          SBUF->SBUF DMA-transpose per (tile, head); o^T = v.T @ P^T;
          hardswish as h=(t-3)*min(t,6)/6 with t=relu(o+bv+3) in f32
          (relu on DVE, min on GpSimd); output projection (bf16, 1/6
          folded into Wp); out written bf16 and upcast on host.
"""
import numpy as np
import ml_dtypes

import concourse.bass as bass
import concourse.mybir as mybir
import concourse.tile as tile
from concourse import bacc
from concourse.bass_utils import run_bass_kernel_spmd

F32 = mybir.dt.float32
BF16 = mybir.dt.bfloat16
AF = mybir.ActivationFunctionType
OP = mybir.AluOpType

B, N, DIM = 32, 784, 512
H, KD, VD, RES = 8, 32, 128, 28
NCORES = 8
BL = B // NCORES          # batches per core = 4
NL = BL * N               # 3136 tokens per core
NT = 7                    # n-tiles per batch (6x128 + 16)
MC = 7                    # m-chunks per batch
EPS = 1e-5


def _rows(i):
    return 128 if i < 6 else 16


def build_nc():
    nc = bacc.Bacc(None, target_bir_lowering=False, debug=False)

    xT = nc.dram_tensor("xT", [DIM, NL], BF16, kind="ExternalInput")
    wqk = nc.dram_tensor("wqk", [DIM, 512], BF16, kind="ExternalInput")
    bqk = nc.dram_tensor("bqk", [128, 4], F32, kind="ExternalInput")
    wv = nc.dram_tensor("wv", [DIM, 1024], BF16, kind="ExternalInput")
    bv3 = nc.dram_tensor("bv3", [128, 8], F32, kind="ExternalInput")
    wp = nc.dram_tensor("wp", [1024, 512], BF16, kind="ExternalInput")
    bp = nc.dram_tensor("bp", [128, 512], F32, kind="ExternalInput")
    eb = nc.dram_tensor("ebias", [H, 896, 784], BF16, kind="ExternalInput")
    out = nc.dram_tensor("out", [NL, 512], BF16, kind="ExternalOutput")

    with tile.TileContext(nc) as tc:
        with (
            tc.tile_pool(name="persist", bufs=1) as pp,
            tc.tile_pool(name="att", bufs=2) as ap_,
            tc.tile_pool(name="ebp", bufs=1) as ebp,
            tc.tile_pool(name="mmps", bufs=2, space="PSUM") as mm_pool,
        ):
            # persistent SBUF tensors
            qkT = pp.tile([128, 4, NL], BF16)      # q/k channels, head-grouped
            v_sb = pp.tile([128, BL, MC, 1024], BF16)
            wp_sb = pp.tile([128, 8, 512], BF16)
            bp_sb = pp.tile([128, 512], F32)
            bv3_sb = pp.tile([128, 8], F32)

            # ---------------- stage 1: projections ----------------
            with tc.tile_pool(name="s1", bufs=1) as s1:
                xT_sb = s1.tile([128, 4, NL], BF16)
                wqk_sb = s1.tile([128, 4, 512], BF16)
                wv_sb = s1.tile([128, 4, 1024], BF16)
                bqk_sb = s1.tile([128, 4], F32)
                xT_r = xT[:].rearrange("(cc p) n -> p cc n", p=128)
                wqk_r = wqk[:].rearrange("(cc p) o -> p cc o", p=128)
                wv_r = wv[:].rearrange("(cc p) o -> p cc o", p=128)
                # merged 3D stripe DMAs: each transfer covers all 4 channel
                # chunks, staying above the per-transfer DMA floor
                nc.sync.dma_start(wqk_sb[:], wqk_r[:])
                nc.sync.dma_start(xT_sb[:, :, 0:448], xT_r[:, :, 0:448])
                nc.sync.dma_start(bqk_sb[:], bqk[:])
                nc.sync.dma_start(bv3_sb[:], bv3[:])
                for ntc in range(1, NL // 448):
                    nc.sync.dma_start(
                        xT_sb[:, :, ntc * 448:(ntc + 1) * 448],
                        xT_r[:, :, ntc * 448:(ntc + 1) * 448])
                for cc in range(4):
                    nc.sync.dma_start(wv_sb[:, cc], wv_r[:, cc])

                # qkT[o, n] accumulation over 4 c-chunks; 448-wide n stripes
                for ntc in range(NL // 448):
                    for oc in range(4):
                        ps = mm_pool.tile([128, 512], F32, tag="mm")
                        for cc in range(4):
                            nc.tensor.matmul(
                                ps[:, :448],
                                wqk_sb[:, cc, oc * 128:(oc + 1) * 128],
                                xT_sb[:, cc, ntc * 448:(ntc + 1) * 448],
                                start=(cc == 0), stop=(cc == 3),
                            )
                        nc.scalar.activation(
                            qkT[:, oc, ntc * 448:(ntc + 1) * 448], ps[:, :448],
                            AF.Identity, bias=bqk_sb[:, oc:oc + 1],
                        )

                # v[n, vd] token-major (raw: bv folded into hardswish bias)
                for b4 in range(BL):
                    for mc in range(6):
                        col0 = b4 * N + mc * 128
                        for vh in range(2):
                            ps = mm_pool.tile([128, 512], F32, tag="mm")
                            for cc in range(4):
                                nc.tensor.matmul(
                                    ps[:],
                                    xT_sb[:, cc, col0:col0 + 128],
                                    wv_sb[:, cc, vh * 512:(vh + 1) * 512],
                                    start=(cc == 0), stop=(cc == 3),
                                )
                            nc.scalar.activation(
                                v_sb[:, b4, mc, vh * 512:(vh + 1) * 512],
                                ps[:], AF.Copy)
                # all 4 batches' 16-row tail tokens in one matmul group:
                # slivers staged 32-partition-aligned with zero padding
                xsl = s1.tile([128, 4, 128], BF16)
                vst = s1.tile([128, 2, 512], BF16)
                nc.vector.memset(xsl[:], 0.0)
                for cc in range(4):
                    nc.vector.tensor_copy(
                        xsl[:, cc].rearrange("p (b j) -> p b j", b=BL)[:, :, :16],
                        xT_sb[:, cc].rearrange(
                            "p (b n) -> p b n", b=BL)[:, :, 768:784])
                for vh in range(2):
                    ps = mm_pool.tile([128, 512], F32, tag="mm")
                    for cc in range(4):
                        nc.tensor.matmul(
                            ps[:],
                            xsl[:, cc],
                            wv_sb[:, cc, vh * 512:(vh + 1) * 512],
                            start=(cc == 0), stop=(cc == 3),
                        )
                    nc.scalar.activation(vst[:, vh], ps[:], AF.Copy)
                for b4 in range(BL):
                    nc.sync.dma_start(
                        v_sb[:16, b4, 6, :],
                        vst[32 * b4:32 * b4 + 16])

            # ---------------- stage 2: attention ----------------
            from functools import partial
            with (
                tc.tile_pool(name="p2", bufs=1) as pp2,
                tc.tile_pool(name="sps", bufs=1, space="PSUM") as s_pool,
            ):
                hT = pp2.tile([128, BL, H, N], BF16)
                # persistent P^T buffers, one per head slot; pipeline sync
                # comes from emission order + region-level WAR/RAW deps
                pt_bufs = [pp2.tile([128, MC, N], BF16, name=f"ptb{i}")
                           for i in range(2)]
                # persistent eb tables, DMA'd chunkwise so the next head
                # pair's table streams in behind the current reads
                ebt = [pp2.tile([128, MC, 784], BF16, name=f"ebt{i}")
                       for i in range(2)]
                # persistent round-robin es buffers (4 per head slot) with a
                # zeroed transpose pad
                es_bufs = [pp2.tile([128, 896], BF16, name=f"esb{j}")
                           for j in range(8)]
                for t in es_bufs:
                    nc.vector.memset(t[:, 784:], 0.0)
                es_cnt = [0, 0]
                s_rot = [0]
                hsl = pp2.tile([128, 8, 128], BF16, name="hsl")
                nc.vector.memset(hsl[:], 0.0)

                def eb_chunk(hp_t, i, mc):
                    ebr = eb[2 * hp_t + i].rearrange(
                        "(mc p) n -> p mc n", p=128)
                    nc.sync.dma_start(ebt[i][:, mc], ebr[:, mc])

                def softmax_tile(hp, b4, nt, defer_tp=None):
                    nr = _rows(nt)
                    c0 = b4 * N + nt * 128
                    nsl = slice(nt * 128, nt * 128 + nr)
                    # 3-way rotating S tiles shared by both heads (global
                    # stream counter): the WAR partner of each S matmul is
                    # the exp 3 stream-slots back, so the S->exp ping-pong
                    # never stalls either chain, including at batch bounds
                    Ss = []
                    for i in range(2):
                        j = s_rot[0] % 3
                        s_rot[0] += 1
                        Ss.append(s_pool.tile([128, 2, 512], F32,
                                              tag=f"sp{j}", name=f"sp{j}"))
                    for i in range(2):
                        h = 2 * hp + i
                        ccq, cck = (0, 1) if h < 4 else (2, 3)
                        pq = 32 * (h % 4)
                        for half in range(2):
                            m0 = b4 * N + half * 392
                            nc.tensor.matmul(
                                Ss[i][:nr, half, :392],
                                qkT[pq:pq + 32, ccq, c0:c0 + nr],
                                qkT[pq:pq + 32, cck, m0:m0 + 392],
                                start=True, stop=True,
                                tile_position=(pq, 0))
                    for i in range(2):
                        es = es_bufs[4 * i + es_cnt[i] % 4]
                        es_cnt[i] += 1
                        den = ap_.tile([128, 1], F32, tag=f"den{i}")
                        nc.scalar.activation(
                            es[:nr, :784].rearrange("p (a b) -> p a b", a=2),
                            Ss[i][:nr, :, :392], AF.Exp)
                        nc.gpsimd.tensor_tensor(
                            es[:nr, :784], es[:nr, :784],
                            ebt[i][:nr, nt, :], OP.mult)
                        nc.vector.tensor_scalar(
                            es[:nr, :784], es[:nr, :784], 1.0, 0.0,
                            OP.mult, OP.add, accum_out=den[:nr])
                        rd = ap_.tile([128, 1], F32, tag=f"rd{i}")
                        nc.vector.reciprocal(rd[:nr], den[:nr])
                        nc.vector.tensor_scalar_mul(
                            es[:nr, :784], es[:nr, :784], rd[:nr, 0:1])
                        tp = partial(nc.sync.dma_start_transpose,
                                     pt_bufs[i][:, :, nsl], es[:nr, :896])
                        if defer_tp is None:
                            tp()
                        else:
                            defer_tp.append(tp)

                def av_group(hp, b4, k):
                    # one (head, col-half) AV + hardswish group of batch b4
                    i = k // 2
                    hs, hw_ = ((0, 448), (448, 336))[k % 2]
                    h = 2 * hp + i
                    ops = mm_pool.tile([128, 512], F32, tag="mm")
                    for mc in range(MC):
                        mr = _rows(mc)
                        nc.tensor.matmul(
                            ops[:, :hw_],
                            v_sb[:mr, b4, mc, h * 128:(h + 1) * 128],
                            pt_bufs[i][:mr, mc, hs:hs + hw_],
                            start=(mc == 0), stop=(mc == 6))
                    tt = ap_.tile([128, 448], F32, tag="hsw")
                    nc.vector.tensor_scalar(
                        tt[:, :hw_], ops[:, :hw_],
                        bv3_sb[:, h:h + 1], 0.0, OP.add, OP.max)
                    mm_ = ap_.tile([128, 448], F32, tag="mmin")
                    nc.gpsimd.tensor_scalar_min(
                        mm_[:, :hw_], tt[:, :hw_], 6.0)
                    nc.vector.scalar_tensor_tensor(
                        hT[:, b4, h, hs:hs + hw_],
                        tt[:, :hw_], -3.0, mm_[:, :hw_],
                        OP.add, OP.mult)

                def proj_group(b4, nt):
                    c0 = b4 * N + nt * 128
                    ps = mm_pool.tile([128, 512], F32, tag="mm")
                    for hh in range(8):
                        nc.tensor.matmul(
                            ps[:],
                            hT[:, b4, hh, nt * 128:nt * 128 + 128],
                            wp_sb[:, hh, :],
                            start=(hh == 0), stop=(hh == 7))
                    ob = ap_.tile([128, 512], BF16, tag="ob")
                    nc.vector.tensor_tensor(ob[:], ps[:], bp_sb[:], OP.add)
                    nc.sync.dma_start(out[c0:c0 + 128, :], ob[:])

                def hsl_copy(hh):
                    nc.vector.tensor_copy(
                        hsl[:, hh].rearrange(
                            "p (b j) -> p b j", b=BL)[:, :, :16],
                        hT[:, :, hh, 768:784])

                def proj_sliver():
                    # 16-row tail tokens of all 4 batches, 32-aligned packing
                    # (heads 0-5 staged early; only the last pair here)
                    for hh in (6, 7):
                        hsl_copy(hh)
                    ps = mm_pool.tile([128, 512], F32, tag="mm")
                    for hh in range(8):
                        nc.tensor.matmul(
                            ps[:], hsl[:, hh], wp_sb[:, hh, :],
                            start=(hh == 0), stop=(hh == 7))
                    obs = ap_.tile([128, 512], BF16, tag="ob")
                    nc.vector.tensor_tensor(obs[:], ps[:], bp_sb[:], OP.add)
                    for b4 in range(BL):
                        nc.sync.dma_start(
                            out[b4 * N + 768:b4 * N + 784, :],
                            obs[32 * b4:32 * b4 + 16])

                # schedule: slot s of batch b4 runs softmax_tile(nt=s) then
                # filler groups, placed >=2 slots after the softmax chain
                # they consume: hs=448 AV groups of the PREVIOUS batch in
                # slots 2-3 (chain of its nt6 ends ~2 slots into this batch;
                # pt cols 448:784 are only overwritten from nt=3 on, emitted
                # after); hs=0 AV groups of the CURRENT batch in slots 5-6
                # (their pt cols 0:448 complete at nt=3); projections of the
                # previous batch follow its hs=448 hardswish in slots 4-6.
                nc.sync.dma_start(
                    wp_sb[:], wp[:].rearrange("(hh p) o -> p hh o", p=128))
                nc.sync.dma_start(bp_sb[:], bp[:])
                prev = None
                for hp in range(4):
                    for b4 in range(BL):
                        slotfill = [[] for _ in range(NT)]
                        if prev is not None:
                            php, pb4 = prev
                            # both hs=448 groups must precede this batch's
                            # nt3 transpose (it overwrites pt cols 448:512);
                            # that transpose is deferred to slot 4, so slot 3
                            # is the last legal (and least stalled) position
                            slotfill[3].append(partial(av_group, php, pb4, 1))
                            slotfill[3].append(partial(av_group, php, pb4, 3))
                            if php == 3:
                                pj = [partial(proj_group, pb4, n_)
                                      for n_ in range(6)]
                                slotfill[4] += pj[0:2]
                                slotfill[5] += pj[2:4]
                                slotfill[6] += pj[4:6]
                            if pb4 == 3:
                                # stage the finished era's hT tail-token
                                # slivers (writers ran in slot 3)
                                slotfill[5].append(
                                    partial(hsl_copy, 2 * php))
                                slotfill[5].append(
                                    partial(hsl_copy, 2 * php + 1))
                        slotfill[5].append(partial(av_group, hp, b4, 0))
                        slotfill[6].append(partial(av_group, hp, b4, 2))
                        for nt in range(NT):
                            # bias-table chunk loads, just-in-time and spread
                            # one chunk pair per slot: hp0 loads chunk nt in
                            # the slot that first reads it; later eras get
                            # chunks 0-3 prefetched at the previous batch 3
                            # and chunks 4-6 early in their own batch 0
                            if hp == 0 and b4 == 0:
                                for i in range(2):
                                    eb_chunk(0, i, nt)
                            elif b4 == 0 and hp > 0 and nt < 3:
                                for i in range(2):
                                    eb_chunk(hp, i, nt + 4)
                            if nt == 3:
                                # run the chain but hold nt3's transposes
                                # until after slot 3's hs=448 AV fillers
                                deferred = []
                                softmax_tile(hp, b4, nt, defer_tp=deferred)
                            else:
                                if nt == 4:
                                    for tp in deferred:
                                        tp()
                                softmax_tile(hp, b4, nt)
                            if b4 == BL - 1 and hp < 3 and nt < 4:
                                for i in range(2):
                                    eb_chunk(hp + 1, i, nt)
                            for f in slotfill[nt]:
                                f()
                        prev = (hp, b4)
                # drain: last batch's hs=448 AV groups + projection; the
                # sliver projection goes first so its small out-DMAs overlap
                # the main projection matmuls instead of trailing them
                av_group(3, 3, 1)
                av_group(3, 3, 3)
                proj_sliver()
                for n_ in range(6):
                    proj_group(3, n_)

    nc.compile()
    return nc


_NC = None


def _prep_weights(qkv_w, qkv_g, qkv_b, qkv_m, qkv_v, ab, proj_w, proj_g,
                  proj_b, proj_m, proj_v, idxs):
    s = qkv_g / np.sqrt(qkv_v + EPS)
    W = qkv_w * s[:, None]                       # [1536, 512]
    bias = qkv_b - qkv_m * s                     # [1536]
    scale = KD ** -0.5
    # head-grouped reorder: chunk0=q0..3, chunk1=k0..3, chunk2=q4..7, chunk3=k4..7
    qk_rows, v_rows = [], []
    for h in range(H):
        base = h * (2 * KD + VD)
        qk_rows.append((np.arange(base, base + KD), True))
        qk_rows.append((np.arange(base + KD, base + 2 * KD), False))
        v_rows.append(np.arange(base + 2 * KD, base + 2 * KD + VD))
    order = []
    for grp in range(4):
        half = grp // 2
        is_q = (grp % 2 == 0)
        for hh in range(4 * half, 4 * half + 4):
            order.append((qk_rows[2 * hh][0] if is_q else qk_rows[2 * hh + 1][0], is_q))
    wqk = np.empty((512, 512), np.float32)
    bqk = np.empty(512, np.float32)
    o = 0
    for rows, is_q in order:
        f = scale if is_q else 1.0
        wqk[:, o:o + KD] = (W[rows] * f).T
        bqk[o:o + KD] = bias[rows] * f
        o += KD
    vr = np.concatenate(v_rows)
    wv = W[vr].T.copy()                          # [512, 1024]
    bv = bias[vr]                                # folded into hardswish bias

    sp = proj_g / np.sqrt(proj_v + EPS)
    # reference: out = h @ proj_w.T * sp + (proj_b - proj_m*sp); fold 1/6 of hswish
    wp = (proj_w * sp[:, None]).T.astype(np.float32) / 6.0   # [1024, 512]
    bp = proj_b - proj_m * sp

    btab = ab[:, idxs].astype(np.float32)                    # [H, 784, 784]
    eb_pad = np.zeros((H, 896, 784), np.float32)
    eb_pad[:, :784] = np.exp(btab)               # multiplicative bias

    return dict(
        wqk=wqk.astype(ml_dtypes.bfloat16),
        bqk=bqk.reshape(4, 128).T.copy(),
        wv=wv.astype(ml_dtypes.bfloat16),
        bv3=(3.0 + bv).reshape(8, 128).T.astype(np.float32).copy(),
        wp=wp.astype(ml_dtypes.bfloat16),
        bp=np.broadcast_to(bp, (128, 512)).astype(np.float32).copy(),
        ebias=eb_pad.astype(ml_dtypes.bfloat16),
    )


def kernel(x, qkv_w, qkv_g, qkv_b, qkv_m, qkv_v, ab,
           proj_w, proj_g, proj_b, proj_m, proj_v, idxs, _trace=False):
    global _NC
    x = np.asarray(x, np.float32)
    shared = _prep_weights(
        np.asarray(qkv_w, np.float32), np.asarray(qkv_g, np.float32),
        np.asarray(qkv_b, np.float32), np.asarray(qkv_m, np.float32),
        np.asarray(qkv_v, np.float32), np.asarray(ab, np.float32),
        np.asarray(proj_w, np.float32), np.asarray(proj_g, np.float32),
        np.asarray(proj_b, np.float32), np.asarray(proj_m, np.float32),
        np.asarray(proj_v, np.float32), np.asarray(idxs))

    if _NC is None:
        _NC = build_nc()
    nc = _NC

    in_maps = []
    for c in range(NCORES):
        xs = x[c * BL:(c + 1) * BL]                      # [4, 784, 512]
        xT = xs.transpose(2, 0, 1).reshape(DIM, NL).astype(ml_dtypes.bfloat16)
        m = dict(shared)
        m["xT"] = xT
        in_maps.append(m)

    res = run_bass_kernel_spmd(nc, in_maps, core_ids=list(range(NCORES)),
                               trace=_trace)
    outs = [res.results[c]["out"].astype(np.float32).reshape(BL, N, DIM)
            for c in range(NCORES)]
    full = np.concatenate(outs, axis=0)
    if _trace:
        return full, res.exec_time_ns
    return full


# revision 46
# speedup vs baseline: 1.0074x; 1.0061x over previous
"""LeViT-style attention (B=32, N=784, DIM=512, H=8, KD=32, VD=128) on 8 trn2 cores.

Strategy: pure data-parallel over batch (4 batches/core, no collectives).
Host folds BN into weights/biases, folds the softmax scale into Wq, and
precomputes EB = exp(rel-pos-bias) (multiplicative form). Per core:
  stage1: qkT = Wqk.T @ xT   (bf16, head-grouped channel layout; bias via
          ACT Identity+bias), v = xT.T @ Wv (token-major, bf16, NO bias:
          since sum_m P[n,m] == 1 the v-bias is folded into the hardswish
          relu bias as (3 + bv[d]) per-partition). The 16-row tail tokens
          of all 4 batches are packed into one 128-wide stationary group
          (32-partition aligned, zero-padded).
  stage2: software-pipelined over (head-pair, batch): softmax of batch b
          overlaps AV/hardswish/projection of batch b-1 (emitted as filler
          groups >=2 slots after the chains they consume; persistent pt
          buffers + emission-order/region WAR semaphores give the sync;
          bias-table chunks stream just-in-time per slot). Per tile:
          S = q.kT (K=32 matmuls, packed row groups, 3-way rotating PSUM
          tiles so the S->exp WAR never stalls); exp on ScalarE (no
          accumulator read); P~ = exp(S)*EB on the Pool engine (TT, SBUF);
          softmax denominator via DVE tensor_scalar accum in 4x mode;
          P normalized on DVE (4x); P^T via a SINGLE padded [nr, 896] bf16
          SBUF->SBUF DMA-transpose per (tile, head); o^T = v.T @ P^T;
          hardswish as h=(t-3)*min(t,6)/6 with t=relu(o+bv+3) in f32
          (relu on DVE, min on GpSimd); output projection (bf16, 1/6
          folded into Wp); out written bf16 and upcast on host.
"""
import numpy as np
import ml_dtypes

import concourse.bass as bass
import concourse.mybir as mybir
import concourse.tile as tile
from concourse import bacc
from concourse.bass_utils import run_bass_kernel_spmd

F32 = mybir.dt.float32
BF16 = mybir.dt.bfloat16
AF = mybir.ActivationFunctionType
OP = mybir.AluOpType

B, N, DIM = 32, 784, 512
H, KD, VD, RES = 8, 32, 128, 28
NCORES = 8
BL = B // NCORES          # batches per core = 4
NL = BL * N               # 3136 tokens per core
NT = 7                    # n-tiles per batch (6x128 + 16)
MC = 7                    # m-chunks per batch
EPS = 1e-5


def _rows(i):
    return 128 if i < 6 else 16


def build_nc():
    nc = bacc.Bacc(None, target_bir_lowering=False, debug=False)

    xT = nc.dram_tensor("xT", [DIM, NL], BF16, kind="ExternalInput")
    wqk = nc.dram_tensor("wqk", [DIM, 512], BF16, kind="ExternalInput")
    bqk = nc.dram_tensor("bqk", [128, 4], F32, kind="ExternalInput")
    wv = nc.dram_tensor("wv", [DIM, 1024], BF16, kind="ExternalInput")
    bv3 = nc.dram_tensor("bv3", [128, 8], F32, kind="ExternalInput")
    wp = nc.dram_tensor("wp", [1024, 512], BF16, kind="ExternalInput")
    bp = nc.dram_tensor("bp", [128, 512], F32, kind="ExternalInput")
    eb = nc.dram_tensor("ebias", [H, 896, 784], BF16, kind="ExternalInput")
    out = nc.dram_tensor("out", [NL, 512], BF16, kind="ExternalOutput")

    with tile.TileContext(nc) as tc:
        with (
            tc.tile_pool(name="persist", bufs=1) as pp,
            tc.tile_pool(name="att", bufs=2) as ap_,
            tc.tile_pool(name="ebp", bufs=1) as ebp,
            tc.tile_pool(name="mmps", bufs=2, space="PSUM") as mm_pool,
        ):
            # persistent SBUF tensors
            qkT = pp.tile([128, 4, NL], BF16)      # q/k channels, head-grouped
            v_sb = pp.tile([128, BL, MC, 1024], BF16)
            wp_sb = pp.tile([128, 8, 512], BF16)
            bp_sb = pp.tile([128, 512], F32)
            bv3_sb = pp.tile([128, 8], F32)

            # ---------------- stage 1: projections ----------------
            with tc.tile_pool(name="s1", bufs=1) as s1:
                xT_sb = s1.tile([128, 4, NL], BF16)
                wqk_sb = s1.tile([128, 4, 512], BF16)
                wv_sb = s1.tile([128, 4, 1024], BF16)
                bqk_sb = s1.tile([128, 4], F32)
                xT_r = xT[:].rearrange("(cc p) n -> p cc n", p=128)
                wqk_r = wqk[:].rearrange("(cc p) o -> p cc o", p=128)
                wv_r = wv[:].rearrange("(cc p) o -> p cc o", p=128)
                # merged 3D stripe DMAs: each transfer covers all 4 channel
                # chunks, staying above the per-transfer DMA floor
                nc.sync.dma_start(wqk_sb[:], wqk_r[:])
                nc.sync.dma_start(xT_sb[:, :, 0:448], xT_r[:, :, 0:448])
                nc.sync.dma_start(bqk_sb[:], bqk[:])
                nc.sync.dma_start(bv3_sb[:], bv3[:])
                for ntc in range(1, NL // 448):
                    nc.sync.dma_start(
                        xT_sb[:, :, ntc * 448:(ntc + 1) * 448],
                        xT_r[:, :, ntc * 448:(ntc + 1) * 448])
                for cc in range(4):
                    nc.sync.dma_start(wv_sb[:, cc], wv_r[:, cc])

                # qkT[o, n] accumulation over 4 c-chunks; 448-wide n stripes
                for ntc in range(NL // 448):
                    for oc in range(4):
                        ps = mm_pool.tile([128, 512], F32, tag="mm")
                        for cc in range(4):
                            nc.tensor.matmul(
                                ps[:, :448],
                                wqk_sb[:, cc, oc * 128:(oc + 1) * 128],
                                xT_sb[:, cc, ntc * 448:(ntc + 1) * 448],
                                start=(cc == 0), stop=(cc == 3),
                            )
                        nc.scalar.activation(
                            qkT[:, oc, ntc * 448:(ntc + 1) * 448], ps[:, :448],
                            AF.Identity, bias=bqk_sb[:, oc:oc + 1],
                        )

                # v[n, vd] token-major (raw: bv folded into hardswish bias)
                for b4 in range(BL):
                    for mc in range(6):
                        col0 = b4 * N + mc * 128
                        for vh in range(2):
                            ps = mm_pool.tile([128, 512], F32, tag="mm")
                            for cc in range(4):
                                nc.tensor.matmul(
                                    ps[:],
                                    xT_sb[:, cc, col0:col0 + 128],
                                    wv_sb[:, cc, vh * 512:(vh + 1) * 512],
                                    start=(cc == 0), stop=(cc == 3),
                                )
                            nc.scalar.activation(
                                v_sb[:, b4, mc, vh * 512:(vh + 1) * 512],
                                ps[:], AF.Copy)
                # all 4 batches' 16-row tail tokens in one matmul group:
                # slivers staged 32-partition-aligned with zero padding
                xsl = s1.tile([128, 4, 128], BF16)
                vst = s1.tile([128, 2, 512], BF16)
                nc.vector.memset(xsl[:], 0.0)
                for cc in range(4):
                    nc.vector.tensor_copy(
                        xsl[:, cc].rearrange("p (b j) -> p b j", b=BL)[:, :, :16],
                        xT_sb[:, cc].rearrange(
                            "p (b n) -> p b n", b=BL)[:, :, 768:784])
                for vh in range(2):
                    ps = mm_pool.tile([128, 512], F32, tag="mm")
                    for cc in range(4):
                        nc.tensor.matmul(
                            ps[:],
                            xsl[:, cc],
                            wv_sb[:, cc, vh * 512:(vh + 1) * 512],
                            start=(cc == 0), stop=(cc == 3),
                        )
                    nc.scalar.activation(vst[:, vh], ps[:], AF.Copy)
                for b4 in range(BL):
                    nc.sync.dma_start(
                        v_sb[:16, b4, 6, :],
                        vst[32 * b4:32 * b4 + 16])

            # ---------------- stage 2: attention ----------------
            from functools import partial
            with (
                tc.tile_pool(name="p2", bufs=1) as pp2,
                tc.tile_pool(name="sps", bufs=1, space="PSUM") as s_pool,
            ):
                hT = pp2.tile([128, BL, H, N], BF16)
                # persistent P^T buffers, one per head slot; pipeline sync
                # comes from emission order + region-level WAR/RAW deps
                pt_bufs = [pp2.tile([128, MC, N], BF16, name=f"ptb{i}")
                           for i in range(2)]
                # persistent eb tables, DMA'd chunkwise so the next head
                # pair's table streams in behind the current reads
                ebt = [pp2.tile([128, MC, 784], BF16, name=f"ebt{i}")
                       for i in range(2)]
                # persistent round-robin es buffers (4 per head slot) with a
                # zeroed transpose pad
                es_bufs = [pp2.tile([128, 896], BF16, name=f"esb{j}")
                           for j in range(8)]
                for t in es_bufs:
                    nc.vector.memset(t[:, 784:], 0.0)
                es_cnt = [0, 0]
                s_rot = [0]
                hsl = pp2.tile([128, 8, 128], BF16, name="hsl")
                nc.vector.memset(hsl[:], 0.0)

                def eb_chunk(hp_t, i, mc):
                    ebr = eb[2 * hp_t + i].rearrange(
                        "(mc p) n -> p mc n", p=128)
                    nc.sync.dma_start(ebt[i][:, mc], ebr[:, mc])

                def softmax_tile(hp, b4, nt, defer_tp=None):
                    nr = _rows(nt)
                    c0 = b4 * N + nt * 128
                    nsl = slice(nt * 128, nt * 128 + nr)
                    # 3-way rotating S tiles shared by both heads (global
                    # stream counter): the WAR partner of each S matmul is
                    # the exp 3 stream-slots back, so the S->exp ping-pong
                    # never stalls either chain, including at batch bounds
                    Ss = []
                    for i in range(2):
                        j = s_rot[0] % 3
                        s_rot[0] += 1
                        Ss.append(s_pool.tile([128, 2, 512], F32,
                                              tag=f"sp{j}", name=f"sp{j}"))
                    for i in range(2):
                        h = 2 * hp + i
                        ccq, cck = (0, 1) if h < 4 else (2, 3)
                        pq = 32 * (h % 4)
                        for half in range(2):
                            m0 = b4 * N + half * 392
                            nc.tensor.matmul(
                                Ss[i][:nr, half, :392],
                                qkT[pq:pq + 32, ccq, c0:c0 + nr],
                                qkT[pq:pq + 32, cck, m0:m0 + 392],
                                start=True, stop=True,
                                tile_position=(pq, 0))
                    for i in range(2):
                        es = es_bufs[4 * i + es_cnt[i] % 4]
                        es_cnt[i] += 1
                        den = ap_.tile([128, 1], F32, tag=f"den{i}")
                        nc.scalar.activation(
                            es[:nr, :784].rearrange("p (a b) -> p a b", a=2),
                            Ss[i][:nr, :, :392], AF.Exp)
                        nc.gpsimd.tensor_tensor(
                            es[:nr, :784], es[:nr, :784],
                            ebt[i][:nr, nt, :], OP.mult)
                        nc.vector.tensor_scalar(
                            es[:nr, :784], es[:nr, :784], 1.0, 0.0,
                            OP.mult, OP.add, accum_out=den[:nr])
                        rd = ap_.tile([128, 1], F32, tag=f"rd{i}")
                        nc.vector.reciprocal(rd[:nr], den[:nr])
                        nc.vector.tensor_scalar_mul(
                            es[:nr, :784], es[:nr, :784], rd[:nr, 0:1])
                        tp = partial(nc.sync.dma_start_transpose,
                                     pt_bufs[i][:, :, nsl], es[:nr, :896])
                        if defer_tp is None:
                            tp()
                        else:
                            defer_tp.append(tp)

                def av_group(hp, b4, k):
                    # one (head, col-half) AV + hardswish group of batch b4
                    i = k // 2
                    hs, hw_ = ((0, 448), (448, 336))[k % 2]
                    h = 2 * hp + i
                    ops = mm_pool.tile([128, 512], F32, tag="mm")
                    for mc in range(MC):
                        mr = _rows(mc)
                        nc.tensor.matmul(
                            ops[:, :hw_],
                            v_sb[:mr, b4, mc, h * 128:(h + 1) * 128],
                            pt_bufs[i][:mr, mc, hs:hs + hw_],
                            start=(mc == 0), stop=(mc == 6))
                    tt = ap_.tile([128, 448], F32, tag="hsw")
                    nc.vector.tensor_scalar(
                        tt[:, :hw_], ops[:, :hw_],
                        bv3_sb[:, h:h + 1], 0.0, OP.add, OP.max)
                    mm_ = ap_.tile([128, 448], F32, tag="mmin")
                    nc.gpsimd.tensor_scalar_min(
                        mm_[:, :hw_], tt[:, :hw_], 6.0)
                    nc.vector.scalar_tensor_tensor(
                        hT[:, b4, h, hs:hs + hw_],
                        tt[:, :hw_], -3.0, mm_[:, :hw_],
                        OP.add, OP.mult)

                def proj_group(b4, nt):
                    c0 = b4 * N + nt * 128
                    ps = mm_pool.tile([128, 512], F32, tag="mm")
                    for hh in range(8):
                        nc.tensor.matmul(
                            ps[:],
                            hT[:, b4, hh, nt * 128:nt * 128 + 128],
                            wp_sb[:, hh, :],
                            start=(hh == 0), stop=(hh == 7))
                    ob = ap_.tile([128, 512], BF16, tag="ob")
                    nc.vector.tensor_tensor(ob[:], ps[:], bp_sb[:], OP.add)
                    nc.sync.dma_start(out[c0:c0 + 128, :], ob[:])

                def hsl_copy(hh):
                    nc.vector.tensor_copy(
                        hsl[:, hh].rearrange(
                            "p (b j) -> p b j", b=BL)[:, :, :16],
                        hT[:, :, hh, 768:784])

                def proj_sliver():
                    # 16-row tail tokens of all 4 batches, 32-aligned packing
                    # (heads 0-5 staged early; only the last pair here)
                    for hh in (6, 7):
                        hsl_copy(hh)
                    ps = mm_pool.tile([128, 512], F32, tag="mm")
                    for hh in range(8):
                        nc.tensor.matmul(
                            ps[:], hsl[:, hh], wp_sb[:, hh, :],
                            start=(hh == 0), stop=(hh == 7))
                    obs = ap_.tile([128, 512], BF16, tag="ob")
                    nc.vector.tensor_tensor(obs[:], ps[:], bp_sb[:], OP.add)
                    for b4 in range(BL):
                        nc.sync.dma_start(
                            out[b4 * N + 768:b4 * N + 784, :],
                            obs[32 * b4:32 * b4 + 16])

                # schedule: slot s of batch b4 runs softmax_tile(nt=s) then
                # filler groups, placed >=2 slots after the softmax chain
                # they consume: hs=448 AV groups of the PREVIOUS batch in
                # slots 2-3 (chain of its nt6 ends ~2 slots into this batch;
                # pt cols 448:784 are only overwritten from nt=3 on, emitted
                # after); hs=0 AV groups of the CURRENT batch in slots 5-6
                # (their pt cols 0:448 complete at nt=3); projections of the
                # previous batch follow its hs=448 hardswish in slots 4-6.
                nc.sync.dma_start(
                    wp_sb[:], wp[:].rearrange("(hh p) o -> p hh o", p=128))
                nc.sync.dma_start(bp_sb[:], bp[:])
                prev = None
                for hp in range(4):
                    for b4 in range(BL):
                        slotfill = [[] for _ in range(NT)]
                        if prev is not None:
                            php, pb4 = prev
                            # both hs=448 groups must precede this batch's
                            # nt3 transpose (it overwrites pt cols 448:512);
                            # that transpose is deferred to slot 4, so slot 3
                            # is the last legal (and least stalled) position
                            slotfill[3].append(partial(av_group, php, pb4, 1))
                            slotfill[3].append(partial(av_group, php, pb4, 3))
                            if php == 3:
                                pj = [partial(proj_group, pb4, n_)
                                      for n_ in range(6)]
                                slotfill[4] += pj[0:2]
                                slotfill[5] += pj[2:4]
                                slotfill[6] += pj[4:6]
                            if pb4 == 3:
                                # stage the finished era's hT tail-token
                                # slivers (writers ran in slot 3)
                                slotfill[5].append(
                                    partial(hsl_copy, 2 * php))
                                slotfill[5].append(
                                    partial(hsl_copy, 2 * php + 1))
                        slotfill[5].append(partial(av_group, hp, b4, 0))
                        slotfill[6].append(partial(av_group, hp, b4, 2))
                        for nt in range(NT):
                            # bias-table chunk loads, just-in-time and spread
                            # one chunk pair per slot: hp0 loads chunk nt in
                            # the slot that first reads it; later eras get
                            # chunks 0-3 prefetched at the previous batch 3
                            # and chunks 4-6 early in their own batch 0
                            if hp == 0 and b4 == 0:
                                for i in range(2):
                                    eb_chunk(0, i, nt)
                            elif b4 == 0 and hp > 0 and 1 <= nt < 4:
                                for i in range(2):
                                    eb_chunk(hp, i, nt + 3)
                            if nt == 3:
                                # run the chain but hold nt3's transposes
                                # until after slot 3's hs=448 AV fillers
                                deferred = []
                                softmax_tile(hp, b4, nt, defer_tp=deferred)
                            else:
                                if nt == 4:
                                    for tp in deferred:
                                        tp()
                                softmax_tile(hp, b4, nt)
                            if b4 == BL - 1 and hp < 3 and nt >= 3:
                                # queue behind this batch's late transposes
                                # (the era-boundary AV fillers wait on them)
                                for i in range(2):
                                    eb_chunk(hp + 1, i, nt - 3)
                            for f in slotfill[nt]:
                                f()
                        prev = (hp, b4)
                # drain: last batch's hs=448 AV groups + projection; the
                # sliver projection goes first so its small out-DMAs overlap
                # the main projection matmuls instead of trailing them
                av_group(3, 3, 1)
                av_group(3, 3, 3)
                proj_sliver()
                for n_ in range(6):
                    proj_group(3, n_)

    nc.compile()
    return nc


_NC = None


def _prep_weights(qkv_w, qkv_g, qkv_b, qkv_m, qkv_v, ab, proj_w, proj_g,
                  proj_b, proj_m, proj_v, idxs):
    s = qkv_g / np.sqrt(qkv_v + EPS)
    W = qkv_w * s[:, None]                       # [1536, 512]
    bias = qkv_b - qkv_m * s                     # [1536]
    scale = KD ** -0.5
    # head-grouped reorder: chunk0=q0..3, chunk1=k0..3, chunk2=q4..7, chunk3=k4..7
    qk_rows, v_rows = [], []
    for h in range(H):
        base = h * (2 * KD + VD)
        qk_rows.append((np.arange(base, base + KD), True))
        qk_rows.append((np.arange(base + KD, base + 2 * KD), False))
        v_rows.append(np.arange(base + 2 * KD, base + 2 * KD + VD))
    order = []
    for grp in range(4):
        half = grp // 2
        is_q = (grp % 2 == 0)
        for hh in range(4 * half, 4 * half + 4):
            order.append((qk_rows[2 * hh][0] if is_q else qk_rows[2 * hh + 1][0], is_q))
    wqk = np.empty((512, 512), np.float32)
    bqk = np.empty(512, np.float32)
    o = 0
    for rows, is_q in order:
        f = scale if is_q else 1.0
        wqk[:, o:o + KD] = (W[rows] * f).T
        bqk[o:o + KD] = bias[rows] * f
        o += KD
    vr = np.concatenate(v_rows)
    wv = W[vr].T.copy()                          # [512, 1024]
    bv = bias[vr]                                # folded into hardswish bias

    sp = proj_g / np.sqrt(proj_v + EPS)
    # reference: out = h @ proj_w.T * sp + (proj_b - proj_m*sp); fold 1/6 of hswish
    wp = (proj_w * sp[:, None]).T.astype(np.float32) / 6.0   # [1024, 512]
    bp = proj_b - proj_m * sp

    btab = ab[:, idxs].astype(np.float32)                    # [H, 784, 784]
    eb_pad = np.zeros((H, 896, 784), np.float32)
    eb_pad[:, :784] = np.exp(btab)               # multiplicative bias

    return dict(
        wqk=wqk.astype(ml_dtypes.bfloat16),
        bqk=bqk.reshape(4, 128).T.copy(),
        wv=wv.astype(ml_dtypes.bfloat16),
        bv3=(3.0 + bv).reshape(8, 128).T.astype(np.float32).copy(),
        wp=wp.astype(ml_dtypes.bfloat16),
        bp=np.broadcast_to(bp, (128, 512)).astype(np.float32).copy(),
        ebias=eb_pad.astype(ml_dtypes.bfloat16),
    )


def kernel(x, qkv_w, qkv_g, qkv_b, qkv_m, qkv_v, ab,
           proj_w, proj_g, proj_b, proj_m, proj_v, idxs, _trace=False):
    global _NC
    x = np.asarray(x, np.float32)
    shared = _prep_weights(
        np.asarray(qkv_w, np.float32), np.asarray(qkv_g, np.float32),
        np.asarray(qkv_b, np.float32), np.asarray(qkv_m, np.float32),
        np.asarray(qkv_v, np.float32), np.asarray(ab, np.float32),
        np.asarray(proj_w, np.float32), np.asarray(proj_g, np.float32),
        np.asarray(proj_b, np.float32), np.asarray(proj_m, np.float32),
        np.asarray(proj_v, np.float32), np.asarray(idxs))

    if _NC is None:
        _NC = build_nc()
    nc = _NC

    in_maps = []
    for c in range(NCORES):
        xs = x[c * BL:(c + 1) * BL]                      # [4, 784, 512]
        xT = xs.transpose(2, 0, 1).reshape(DIM, NL).astype(ml_dtypes.bfloat16)
        m = dict(shared)
        m["xT"] = xT
        in_maps.append(m)

    res = run_bass_kernel_spmd(nc, in_maps, core_ids=list(range(NCORES)),
                               trace=_trace)
    outs = [res.results[c]["out"].astype(np.float32).reshape(BL, N, DIM)
            for c in range(NCORES)]
    full = np.concatenate(outs, axis=0)
    if _trace:
        return full, res.exec_time_ns
    return full


# revision 52
# speedup vs baseline: 1.0104x; 1.0030x over previous
"""LeViT-style attention (B=32, N=784, DIM=512, H=8, KD=32, VD=128) on 8 trn2 cores.

Strategy: pure data-parallel over batch (4 batches/core, no collectives).
Host folds BN into weights/biases, folds the softmax scale into Wq, and
precomputes EB = exp(rel-pos-bias) (multiplicative form). Per core:
  stage1: qkT = Wqk.T @ xT   (bf16, head-grouped channel layout; bias via
          ACT Identity+bias), v = xT.T @ Wv (token-major, bf16, NO bias:
          since sum_m P[n,m] == 1 the v-bias is folded into the hardswish
          relu bias as (3 + bv[d]) per-partition). The 16-row tail tokens
          of all 4 batches are packed into one 128-wide stationary group
          (32-partition aligned, zero-padded).
  stage2: software-pipelined over (head-pair, batch): softmax of batch b
          overlaps AV/hardswish/projection of batch b-1 (emitted as filler
          groups >=2 slots after the chains they consume; persistent pt
          buffers + emission-order/region WAR semaphores give the sync;
          bias-table chunks stream just-in-time per slot). Per tile:
          S = q.kT (K=32 matmuls, packed row groups, 3-way rotating PSUM
          tiles so the S->exp WAR never stalls); exp on ScalarE (no
          accumulator read); P~ = exp(S)*EB on the Pool engine (TT, SBUF);
          softmax denominator via DVE tensor_scalar accum in 4x mode;
          P normalized on DVE (4x); P^T via a SINGLE padded [nr, 896] bf16
          SBUF->SBUF DMA-transpose per (tile, head); o^T = v.T @ P^T;
          hardswish as h=(t-3)*min(t,6)/6 with t=relu(o+bv+3) in f32
          (relu on DVE, min on GpSimd); output projection (bf16, 1/6
          folded into Wp); out written bf16 and upcast on host.
"""
import numpy as np
import ml_dtypes

import concourse.bass as bass
import concourse.mybir as mybir
import concourse.tile as tile
from concourse import bacc
from concourse.bass_utils import run_bass_kernel_spmd

F32 = mybir.dt.float32
BF16 = mybir.dt.bfloat16
AF = mybir.ActivationFunctionType
OP = mybir.AluOpType

B, N, DIM = 32, 784, 512
H, KD, VD, RES = 8, 32, 128, 28
NCORES = 8
BL = B // NCORES          # batches per core = 4
NL = BL * N               # 3136 tokens per core
NT = 7                    # n-tiles per batch (6x128 + 16)
MC = 7                    # m-chunks per batch
EPS = 1e-5


def _rows(i):
    return 128 if i < 6 else 16


def build_nc():
    nc = bacc.Bacc(None, target_bir_lowering=False, debug=False)

    xT = nc.dram_tensor("xT", [DIM, NL], BF16, kind="ExternalInput")
    wqk = nc.dram_tensor("wqk", [DIM, 512], BF16, kind="ExternalInput")
    bqk = nc.dram_tensor("bqk", [128, 4], F32, kind="ExternalInput")
    wv = nc.dram_tensor("wv", [DIM, 1024], BF16, kind="ExternalInput")
    bv3 = nc.dram_tensor("bv3", [128, 8], F32, kind="ExternalInput")
    wp = nc.dram_tensor("wp", [1024, 512], BF16, kind="ExternalInput")
    bp = nc.dram_tensor("bp", [128, 512], F32, kind="ExternalInput")
    eb = nc.dram_tensor("ebias", [H, 896, 784], BF16, kind="ExternalInput")
    out = nc.dram_tensor("out", [NL, 512], BF16, kind="ExternalOutput")

    with tile.TileContext(nc) as tc:
        with (
            tc.tile_pool(name="persist", bufs=1) as pp,
            tc.tile_pool(name="att", bufs=2) as ap_,
            tc.tile_pool(name="ebp", bufs=1) as ebp,
            tc.tile_pool(name="mmps", bufs=2, space="PSUM") as mm_pool,
        ):
            # persistent SBUF tensors
            qkT = pp.tile([128, 4, NL], BF16)      # q/k channels, head-grouped
            v_sb = pp.tile([128, BL, MC, 1024], BF16)
            wp_sb = pp.tile([128, 8, 512], BF16)
            bp_sb = pp.tile([128, 512], F32)
            bv3_sb = pp.tile([128, 8], F32)
            # persistent eb tables (loaded chunkwise; hp0's full load rides
            # the idle stage-1 DMA stream)
            ebt = [pp.tile([128, MC, 784], BF16, name=f"ebt{i}")
                   for i in range(2)]

            def eb_chunk(hp_t, i, mc):
                ebr = eb[2 * hp_t + i].rearrange("(mc p) n -> p mc n", p=128)
                nc.sync.dma_start(ebt[i][:, mc], ebr[:, mc])

            # ---------------- stage 1: projections ----------------
            with tc.tile_pool(name="s1", bufs=1) as s1:
                xT_sb = s1.tile([128, 4, NL], BF16)
                wqk_sb = s1.tile([128, 4, 512], BF16)
                wv_sb = s1.tile([128, 4, 1024], BF16)
                bqk_sb = s1.tile([128, 4], F32)
                xT_r = xT[:].rearrange("(cc p) n -> p cc n", p=128)
                wqk_r = wqk[:].rearrange("(cc p) o -> p cc o", p=128)
                wv_r = wv[:].rearrange("(cc p) o -> p cc o", p=128)
                # merged 3D stripe DMAs: each transfer covers all 4 channel
                # chunks, staying above the per-transfer DMA floor
                nc.sync.dma_start(wqk_sb[:], wqk_r[:])
                nc.sync.dma_start(xT_sb[:, :, 0:448], xT_r[:, :, 0:448])
                nc.sync.dma_start(bqk_sb[:], bqk[:])
                nc.sync.dma_start(bv3_sb[:], bv3[:])
                for ntc in range(1, NL // 448):
                    nc.sync.dma_start(
                        xT_sb[:, :, ntc * 448:(ntc + 1) * 448],
                        xT_r[:, :, ntc * 448:(ntc + 1) * 448])
                for cc in range(4):
                    nc.sync.dma_start(wv_sb[:, cc], wv_r[:, cc])
                # late-use inputs ride the idle stage-1 DMA stream so the
                # first attention cycles' transposes get a clean queue
                nc.sync.dma_start(
                    wp_sb[:], wp[:].rearrange("(hh p) o -> p hh o", p=128))
                nc.sync.dma_start(bp_sb[:], bp[:])
                for mc in range(MC):
                    for i in range(2):
                        eb_chunk(0, i, mc)

                # qkT[o, n] accumulation over 4 c-chunks; 448-wide n stripes
                for ntc in range(NL // 448):
                    for oc in range(4):
                        ps = mm_pool.tile([128, 512], F32, tag="mm")
                        for cc in range(4):
                            nc.tensor.matmul(
                                ps[:, :448],
                                wqk_sb[:, cc, oc * 128:(oc + 1) * 128],
                                xT_sb[:, cc, ntc * 448:(ntc + 1) * 448],
                                start=(cc == 0), stop=(cc == 3),
                            )
                        nc.scalar.activation(
                            qkT[:, oc, ntc * 448:(ntc + 1) * 448], ps[:, :448],
                            AF.Identity, bias=bqk_sb[:, oc:oc + 1],
                        )

                # v[n, vd] token-major (raw: bv folded into hardswish bias)
                for b4 in range(BL):
                    for mc in range(6):
                        col0 = b4 * N + mc * 128
                        for vh in range(2):
                            ps = mm_pool.tile([128, 512], F32, tag="mm")
                            for cc in range(4):
                                nc.tensor.matmul(
                                    ps[:],
                                    xT_sb[:, cc, col0:col0 + 128],
                                    wv_sb[:, cc, vh * 512:(vh + 1) * 512],
                                    start=(cc == 0), stop=(cc == 3),
                                )
                            nc.scalar.activation(
                                v_sb[:, b4, mc, vh * 512:(vh + 1) * 512],
                                ps[:], AF.Copy)
                # all 4 batches' 16-row tail tokens in one matmul group:
                # slivers staged 32-partition-aligned with zero padding
                xsl = s1.tile([128, 4, 128], BF16)
                vst = s1.tile([128, 2, 512], BF16)
                nc.vector.memset(xsl[:], 0.0)
                for cc in range(4):
                    nc.vector.tensor_copy(
                        xsl[:, cc].rearrange("p (b j) -> p b j", b=BL)[:, :, :16],
                        xT_sb[:, cc].rearrange(
                            "p (b n) -> p b n", b=BL)[:, :, 768:784])
                for vh in range(2):
                    ps = mm_pool.tile([128, 512], F32, tag="mm")
                    for cc in range(4):
                        nc.tensor.matmul(
                            ps[:],
                            xsl[:, cc],
                            wv_sb[:, cc, vh * 512:(vh + 1) * 512],
                            start=(cc == 0), stop=(cc == 3),
                        )
                    nc.scalar.activation(vst[:, vh], ps[:], AF.Copy)
                for b4 in range(BL):
                    nc.sync.dma_start(
                        v_sb[:16, b4, 6, :],
                        vst[32 * b4:32 * b4 + 16])

            # ---------------- stage 2: attention ----------------
            from functools import partial
            with (
                tc.tile_pool(name="p2", bufs=1) as pp2,
                tc.tile_pool(name="sps", bufs=1, space="PSUM") as s_pool,
            ):
                hT = pp2.tile([128, BL, H, N], BF16)
                # persistent P^T buffers, one per head slot; pipeline sync
                # comes from emission order + region-level WAR/RAW deps
                pt_bufs = [pp2.tile([128, MC, N], BF16, name=f"ptb{i}")
                           for i in range(2)]
                # persistent round-robin es buffers (4 per head slot) with a
                # zeroed transpose pad
                es_bufs = [pp2.tile([128, 896], BF16, name=f"esb{j}")
                           for j in range(8)]
                for t in es_bufs:
                    nc.vector.memset(t[:, 784:], 0.0)
                es_cnt = [0, 0]
                s_rot = [0]
                hsl = pp2.tile([128, 8, 128], BF16, name="hsl")
                nc.vector.memset(hsl[:], 0.0)

                def softmax_tile(hp, b4, nt, defer_tp=None):
                    nr = _rows(nt)
                    c0 = b4 * N + nt * 128
                    nsl = slice(nt * 128, nt * 128 + nr)
                    # 3-way rotating S tiles shared by both heads (global
                    # stream counter): the WAR partner of each S matmul is
                    # the exp 3 stream-slots back, so the S->exp ping-pong
                    # never stalls either chain, including at batch bounds
                    Ss = []
                    for i in range(2):
                        j = s_rot[0] % 3
                        s_rot[0] += 1
                        Ss.append(s_pool.tile([128, 2, 512], F32,
                                              tag=f"sp{j}", name=f"sp{j}"))
                    for i in range(2):
                        h = 2 * hp + i
                        ccq, cck = (0, 1) if h < 4 else (2, 3)
                        pq = 32 * (h % 4)
                        for half in range(2):
                            m0 = b4 * N + half * 392
                            nc.tensor.matmul(
                                Ss[i][:nr, half, :392],
                                qkT[pq:pq + 32, ccq, c0:c0 + nr],
                                qkT[pq:pq + 32, cck, m0:m0 + 392],
                                start=True, stop=True,
                                tile_position=(pq, 0))
                    for i in range(2):
                        es = es_bufs[4 * i + es_cnt[i] % 4]
                        es_cnt[i] += 1
                        den = ap_.tile([128, 1], F32, tag=f"den{i}")
                        nc.scalar.activation(
                            es[:nr, :784].rearrange("p (a b) -> p a b", a=2),
                            Ss[i][:nr, :, :392], AF.Exp)
                        nc.gpsimd.tensor_tensor(
                            es[:nr, :784], es[:nr, :784],
                            ebt[i][:nr, nt, :], OP.mult)
                        nc.vector.tensor_scalar(
                            es[:nr, :784], es[:nr, :784], 1.0, 0.0,
                            OP.mult, OP.add, accum_out=den[:nr])
                        rd = ap_.tile([128, 1], F32, tag=f"rd{i}")
                        nc.vector.reciprocal(rd[:nr], den[:nr])
                        nc.vector.tensor_scalar_mul(
                            es[:nr, :784], es[:nr, :784], rd[:nr, 0:1])
                        tp = partial(nc.sync.dma_start_transpose,
                                     pt_bufs[i][:, :, nsl], es[:nr, :896])
                        if defer_tp is None:
                            tp()
                        else:
                            defer_tp.append(tp)

                def av_group(hp, b4, k):
                    # one (head, col-half) AV + hardswish group of batch b4
                    i = k // 2
                    hs, hw_ = ((0, 448), (448, 336))[k % 2]
                    h = 2 * hp + i
                    ops = mm_pool.tile([128, 512], F32, tag="mm")
                    for mc in range(MC):
                        mr = _rows(mc)
                        nc.tensor.matmul(
                            ops[:, :hw_],
                            v_sb[:mr, b4, mc, h * 128:(h + 1) * 128],
                            pt_bufs[i][:mr, mc, hs:hs + hw_],
                            start=(mc == 0), stop=(mc == 6))
                    tt = ap_.tile([128, 448], F32, tag="hsw")
                    nc.vector.tensor_scalar(
                        tt[:, :hw_], ops[:, :hw_],
                        bv3_sb[:, h:h + 1], 0.0, OP.add, OP.max)
                    mm_ = ap_.tile([128, 448], F32, tag="mmin")
                    nc.gpsimd.tensor_scalar_min(
                        mm_[:, :hw_], tt[:, :hw_], 6.0)
                    nc.vector.scalar_tensor_tensor(
                        hT[:, b4, h, hs:hs + hw_],
                        tt[:, :hw_], -3.0, mm_[:, :hw_],
                        OP.add, OP.mult)

                def proj_group(b4, nt):
                    c0 = b4 * N + nt * 128
                    ps = mm_pool.tile([128, 512], F32, tag="mm")
                    for hh in range(8):
                        nc.tensor.matmul(
                            ps[:],
                            hT[:, b4, hh, nt * 128:nt * 128 + 128],
                            wp_sb[:, hh, :],
                            start=(hh == 0), stop=(hh == 7))
                    ob = ap_.tile([128, 512], BF16, tag="ob")
                    nc.vector.tensor_tensor(ob[:], ps[:], bp_sb[:], OP.add)
                    nc.sync.dma_start(out[c0:c0 + 128, :], ob[:])

                def hsl_copy(hh):
                    nc.vector.tensor_copy(
                        hsl[:, hh].rearrange(
                            "p (b j) -> p b j", b=BL)[:, :, :16],
                        hT[:, :, hh, 768:784])

                def proj_sliver():
                    # 16-row tail tokens of all 4 batches, 32-aligned packing
                    # (heads 0-5 staged early; only the last pair here)
                    for hh in (6, 7):
                        hsl_copy(hh)
                    ps = mm_pool.tile([128, 512], F32, tag="mm")
                    for hh in range(8):
                        nc.tensor.matmul(
                            ps[:], hsl[:, hh], wp_sb[:, hh, :],
                            start=(hh == 0), stop=(hh == 7))
                    obs = ap_.tile([128, 512], BF16, tag="ob")
                    nc.vector.tensor_tensor(obs[:], ps[:], bp_sb[:], OP.add)
                    for b4 in range(BL):
                        nc.sync.dma_start(
                            out[b4 * N + 768:b4 * N + 784, :],
                            obs[32 * b4:32 * b4 + 16])

                # schedule: slot s of batch b4 runs softmax_tile(nt=s) then
                # filler groups, placed >=2 slots after the softmax chain
                # they consume: hs=448 AV groups of the PREVIOUS batch in
                # slots 2-3 (chain of its nt6 ends ~2 slots into this batch;
                # pt cols 448:784 are only overwritten from nt=3 on, emitted
                # after); hs=0 AV groups of the CURRENT batch in slots 5-6
                # (their pt cols 0:448 complete at nt=3); projections of the
                # previous batch follow its hs=448 hardswish in slots 4-6.
                prev = None
                for hp in range(4):
                    for b4 in range(BL):
                        slotfill = [[] for _ in range(NT)]
                        if prev is not None:
                            php, pb4 = prev
                            # both hs=448 groups must precede this batch's
                            # nt3 transpose (it overwrites pt cols 448:512);
                            # that transpose is deferred to slot 4, so slot 3
                            # is the last legal (and least stalled) position
                            slotfill[3].append(partial(av_group, php, pb4, 1))
                            slotfill[3].append(partial(av_group, php, pb4, 3))
                            if php == 3:
                                pj = [partial(proj_group, pb4, n_)
                                      for n_ in range(6)]
                                slotfill[4] += pj[0:2]
                                slotfill[5] += pj[2:4]
                                slotfill[6] += pj[4:6]
                            if pb4 == 3:
                                # stage the finished era's hT tail-token
                                # slivers (writers ran in slot 3)
                                slotfill[5].append(
                                    partial(hsl_copy, 2 * php))
                                slotfill[5].append(
                                    partial(hsl_copy, 2 * php + 1))
                        slotfill[5].append(partial(av_group, hp, b4, 0))
                        slotfill[6].append(partial(av_group, hp, b4, 2))
                        for nt in range(NT):
                            # bias-table chunk loads (hp0's full table rode
                            # the stage-1 stream): chunks 0-3 prefetched at
                            # the previous batch 3, chunks 4-6 early in the
                            # era's own batch 0, queued behind the
                            # boundary-critical transposes
                            if b4 == 0 and hp > 0 and 1 <= nt < 4:
                                for i in range(2):
                                    eb_chunk(hp, i, nt + 3)
                            if nt == 3:
                                # run the chain but hold nt3's transposes
                                # until after slot 3's hs=448 AV fillers
                                deferred = []
                                softmax_tile(hp, b4, nt, defer_tp=deferred)
                            else:
                                if nt == 4:
                                    for tp in deferred:
                                        tp()
                                softmax_tile(hp, b4, nt)
                            if b4 == BL - 1 and hp < 3 and nt >= 3:
                                # queue behind this batch's late transposes
                                # (the era-boundary AV fillers wait on them)
                                for i in range(2):
                                    eb_chunk(hp + 1, i, nt - 3)
                            for f in slotfill[nt]:
                                f()
                        prev = (hp, b4)
                # drain: last batch's hs=448 AV groups + projection; the
                # sliver projection goes first so its small out-DMAs overlap
                # the main projection matmuls instead of trailing them
                av_group(3, 3, 1)
                av_group(3, 3, 3)
                proj_sliver()
                for n_ in range(6):
                    proj_group(3, n_)

    nc.compile()
    return nc


_NC = None


def _prep_weights(qkv_w, qkv_g, qkv_b, qkv_m, qkv_v, ab, proj_w, proj_g,
                  proj_b, proj_m, proj_v, idxs):
    s = qkv_g / np.sqrt(qkv_v + EPS)
    W = qkv_w * s[:, None]                       # [1536, 512]
    bias = qkv_b - qkv_m * s                     # [1536]
    scale = KD ** -0.5
    # head-grouped reorder: chunk0=q0..3, chunk1=k0..3, chunk2=q4..7, chunk3=k4..7
    qk_rows, v_rows = [], []
    for h in range(H):
        base = h * (2 * KD + VD)
        qk_rows.append((np.arange(base, base + KD), True))
        qk_rows.append((np.arange(base + KD, base + 2 * KD), False))
        v_rows.append(np.arange(base + 2 * KD, base + 2 * KD + VD))
    order = []
    for grp in range(4):
        half = grp // 2
        is_q = (grp % 2 == 0)
        for hh in range(4 * half, 4 * half + 4):
            order.append((qk_rows[2 * hh][0] if is_q else qk_rows[2 * hh + 1][0], is_q))
    wqk = np.empty((512, 512), np.float32)
    bqk = np.empty(512, np.float32)
    o = 0
    for rows, is_q in order:
        f = scale if is_q else 1.0
        wqk[:, o:o + KD] = (W[rows] * f).T
        bqk[o:o + KD] = bias[rows] * f
        o += KD
    vr = np.concatenate(v_rows)
    wv = W[vr].T.copy()                          # [512, 1024]
    bv = bias[vr]                                # folded into hardswish bias

    sp = proj_g / np.sqrt(proj_v + EPS)
    # reference: out = h @ proj_w.T * sp + (proj_b - proj_m*sp); fold 1/6 of hswish
    wp = (proj_w * sp[:, None]).T.astype(np.float32) / 6.0   # [1024, 512]
    bp = proj_b - proj_m * sp

    btab = ab[:, idxs].astype(np.float32)                    # [H, 784, 784]
    eb_pad = np.zeros((H, 896, 784), np.float32)
    eb_pad[:, :784] = np.exp(btab)               # multiplicative bias

    return dict(
        wqk=wqk.astype(ml_dtypes.bfloat16),
        bqk=bqk.reshape(4, 128).T.copy(),
        wv=wv.astype(ml_dtypes.bfloat16),
        bv3=(3.0 + bv).reshape(8, 128).T.astype(np.float32).copy(),
        wp=wp.astype(ml_dtypes.bfloat16),
        bp=np.broadcast_to(bp, (128, 512)).astype(np.float32).copy(),
        ebias=eb_pad.astype(ml_dtypes.bfloat16),
    )


def kernel(x, qkv_w, qkv_g, qkv_b, qkv_m, qkv_v, ab,
           proj_w, proj_g, proj_b, proj_m, proj_v, idxs, _trace=False):
    global _NC
    x = np.asarray(x, np.float32)
    shared = _prep_weights(
        np.asarray(qkv_w, np.float32), np.asarray(qkv_g, np.float32),
        np.asarray(qkv_b, np.float32), np.asarray(qkv_m, np.float32),
        np.asarray(qkv_v, np.float32), np.asarray(ab, np.float32),
        np.asarray(proj_w, np.float32), np.asarray(proj_g, np.float32),
        np.asarray(proj_b, np.float32), np.asarray(proj_m, np.float32),
        np.asarray(proj_v, np.float32), np.asarray(idxs))

    if _NC is None:
        _NC = build_nc()
    nc = _NC

    in_maps = []
    for c in range(NCORES):
        xs = x[c * BL:(c + 1) * BL]                      # [4, 784, 512]
        xT = xs.transpose(2, 0, 1).reshape(DIM, NL).astype(ml_dtypes.bfloat16)
        m = dict(shared)
        m["xT"] = xT
        in_maps.append(m)

    res = run_bass_kernel_spmd(nc, in_maps, core_ids=list(range(NCORES)),
                               trace=_trace)
    outs = [res.results[c]["out"].astype(np.float32).reshape(BL, N, DIM)
            for c in range(NCORES)]
    full = np.concatenate(outs, axis=0)
    if _trace:
        return full, res.exec_time_ns
    return full


# revision 54
# speedup vs baseline: 1.0261x; 1.0155x over previous
"""LeViT-style attention (B=32, N=784, DIM=512, H=8, KD=32, VD=128) on 8 trn2 cores.

Strategy: pure data-parallel over batch (4 batches/core, no collectives).
Host folds BN into weights/biases, folds the softmax scale into Wq, and
precomputes EB = exp(rel-pos-bias) (multiplicative form). Per core:
  stage1: qkT = Wqk.T @ xT   (bf16, head-grouped channel layout; bias via
          ACT Identity+bias), v = xT.T @ Wv (token-major, bf16, NO bias:
          since sum_m P[n,m] == 1 the v-bias is folded into the hardswish
          relu bias as (3 + bv[d]) per-partition). The 16-row tail tokens
          of all 4 batches are packed into one 128-wide stationary group
          (32-partition aligned, zero-padded).
  stage2: software-pipelined over (head-pair, batch): softmax of batch b
          overlaps AV/hardswish/projection of batch b-1 (emitted as filler
          groups >=2 slots after the chains they consume; persistent pt
          buffers + emission-order/region WAR semaphores give the sync;
          bias-table chunks stream just-in-time per slot). Per tile:
          S = q.kT (K=32 matmuls, packed row groups, 3-way rotating PSUM
          tiles so the S->exp WAR never stalls); exp on ScalarE (no
          accumulator read); P~ = exp(S)*EB on the Pool engine (TT, SBUF);
          softmax denominator via DVE tensor_scalar accum in 4x mode;
          P normalized on DVE (4x); P^T via a SINGLE padded [nr, 896] bf16
          SBUF->SBUF DMA-transpose per (tile, head); o^T = v.T @ P^T;
          hardswish as h=(t-3)*min(t,6)/6 with t=relu(o+bv+3) in f32
          (relu on DVE, min on GpSimd); output projection (bf16, 1/6
          folded into Wp); out written bf16 and upcast on host.
"""
import numpy as np
import ml_dtypes

import concourse.bass as bass
import concourse.mybir as mybir
import concourse.tile as tile
from concourse import bacc
from concourse.bass_utils import run_bass_kernel_spmd

F32 = mybir.dt.float32
BF16 = mybir.dt.bfloat16
AF = mybir.ActivationFunctionType
OP = mybir.AluOpType

B, N, DIM = 32, 784, 512
H, KD, VD, RES = 8, 32, 128, 28
NCORES = 8
BL = B // NCORES          # batches per core = 4
NL = BL * N               # 3136 tokens per core
NT = 7                    # n-tiles per batch (6x128 + 16)
MC = 7                    # m-chunks per batch
EPS = 1e-5


def _rows(i):
    return 128 if i < 6 else 16


def build_nc():
    nc = bacc.Bacc(None, target_bir_lowering=False, debug=False)

    xT = nc.dram_tensor("xT", [DIM, NL], BF16, kind="ExternalInput")
    wqk = nc.dram_tensor("wqk", [DIM, 512], BF16, kind="ExternalInput")
    bqk = nc.dram_tensor("bqk", [128, 4], F32, kind="ExternalInput")
    wv = nc.dram_tensor("wv", [DIM, 1024], BF16, kind="ExternalInput")
    bv3 = nc.dram_tensor("bv3", [128, 8], F32, kind="ExternalInput")
    wp = nc.dram_tensor("wp", [1024, 512], BF16, kind="ExternalInput")
    bp = nc.dram_tensor("bp", [128, 512], F32, kind="ExternalInput")
    eb = nc.dram_tensor("ebias", [H, 896, 784], BF16, kind="ExternalInput")
    out = nc.dram_tensor("out", [NL, 512], BF16, kind="ExternalOutput")

    with tile.TileContext(nc) as tc:
        with (
            tc.tile_pool(name="persist", bufs=1) as pp,
            tc.tile_pool(name="att", bufs=2) as ap_,
            tc.tile_pool(name="ebp", bufs=1) as ebp,
            tc.tile_pool(name="mmps", bufs=2, space="PSUM") as mm_pool,
        ):
            # persistent SBUF tensors
            qkT = pp.tile([128, 4, NL], BF16)      # q/k channels, head-grouped
            v_sb = pp.tile([128, BL, MC, 1024], BF16)
            wp_sb = pp.tile([128, 8, 512], BF16)
            bp_sb = pp.tile([128, 512], F32)
            bv3_sb = pp.tile([128, 8], F32)
            # persistent eb tables (loaded chunkwise; hp0's full load rides
            # the idle stage-1 DMA stream)
            ebt = [pp.tile([128, MC, 784], BF16, name=f"ebt{i}")
                   for i in range(2)]

            def eb_chunk(hp_t, i, mc):
                ebr = eb[2 * hp_t + i].rearrange("(mc p) n -> p mc n", p=128)
                nc.sync.dma_start(ebt[i][:, mc], ebr[:, mc])

            # ---------------- stage 1: projections ----------------
            with tc.tile_pool(name="s1", bufs=1) as s1:
                xT_sb = s1.tile([128, 4, NL], BF16)
                wqk_sb = s1.tile([128, 4, 512], BF16)
                wv_sb = s1.tile([128, 4, 1024], BF16)
                bqk_sb = s1.tile([128, 4], F32)
                xT_r = xT[:].rearrange("(cc p) n -> p cc n", p=128)
                wqk_r = wqk[:].rearrange("(cc p) o -> p cc o", p=128)
                wv_r = wv[:].rearrange("(cc p) o -> p cc o", p=128)
                # merged 3D stripe DMAs: each transfer covers all 4 channel
                # chunks, staying above the per-transfer DMA floor
                nc.sync.dma_start(wqk_sb[:], wqk_r[:])
                nc.sync.dma_start(xT_sb[:, :, 0:448], xT_r[:, :, 0:448])
                nc.sync.dma_start(bqk_sb[:], bqk[:])
                nc.sync.dma_start(bv3_sb[:], bv3[:])
                for ntc in range(1, NL // 448):
                    nc.sync.dma_start(
                        xT_sb[:, :, ntc * 448:(ntc + 1) * 448],
                        xT_r[:, :, ntc * 448:(ntc + 1) * 448])
                for cc in range(4):
                    nc.sync.dma_start(wv_sb[:, cc], wv_r[:, cc])
                # late-use inputs ride the idle stage-1 DMA stream so the
                # first attention cycles' transposes get a clean queue
                nc.sync.dma_start(
                    wp_sb[:], wp[:].rearrange("(hh p) o -> p hh o", p=128))
                nc.sync.dma_start(bp_sb[:], bp[:])
                for mc in range(MC):
                    for i in range(2):
                        eb_chunk(0, i, mc)

                # qkT[o, n] accumulation over 4 c-chunks; 448-wide n stripes
                for ntc in range(NL // 448):
                    for oc in range(4):
                        ps = mm_pool.tile([128, 512], F32, tag="mm")
                        for cc in range(4):
                            nc.tensor.matmul(
                                ps[:, :448],
                                wqk_sb[:, cc, oc * 128:(oc + 1) * 128],
                                xT_sb[:, cc, ntc * 448:(ntc + 1) * 448],
                                start=(cc == 0), stop=(cc == 3),
                            )
                        nc.scalar.activation(
                            qkT[:, oc, ntc * 448:(ntc + 1) * 448], ps[:, :448],
                            AF.Identity, bias=bqk_sb[:, oc:oc + 1],
                        )

                # all 4 batches' 16-row tail tokens in one matmul group,
                # emitted before the main v loop so the sliver-distribution
                # DMAs clear the queue well before stage 2's transposes
                xsl = s1.tile([128, 4, 128], BF16)
                vst = s1.tile([128, 2, 512], BF16)
                nc.vector.memset(xsl[:], 0.0)
                for cc in range(4):
                    nc.vector.tensor_copy(
                        xsl[:, cc].rearrange("p (b j) -> p b j", b=BL)[:, :, :16],
                        xT_sb[:, cc].rearrange(
                            "p (b n) -> p b n", b=BL)[:, :, 768:784])
                for vh in range(2):
                    ps = mm_pool.tile([128, 512], F32, tag="mm")
                    for cc in range(4):
                        nc.tensor.matmul(
                            ps[:],
                            xsl[:, cc],
                            wv_sb[:, cc, vh * 512:(vh + 1) * 512],
                            start=(cc == 0), stop=(cc == 3),
                        )
                    nc.scalar.activation(vst[:, vh], ps[:], AF.Copy)
                for b4 in range(BL):
                    nc.sync.dma_start(
                        v_sb[:16, b4, 6, :],
                        vst[32 * b4:32 * b4 + 16])

                # v[n, vd] token-major (raw: bv folded into hardswish bias)
                for b4 in range(BL):
                    for mc in range(6):
                        col0 = b4 * N + mc * 128
                        for vh in range(2):
                            ps = mm_pool.tile([128, 512], F32, tag="mm")
                            for cc in range(4):
                                nc.tensor.matmul(
                                    ps[:],
                                    xT_sb[:, cc, col0:col0 + 128],
                                    wv_sb[:, cc, vh * 512:(vh + 1) * 512],
                                    start=(cc == 0), stop=(cc == 3),
                                )
                            nc.scalar.activation(
                                v_sb[:, b4, mc, vh * 512:(vh + 1) * 512],
                                ps[:], AF.Copy)
            # ---------------- stage 2: attention ----------------
            from functools import partial
            with (
                tc.tile_pool(name="p2", bufs=1) as pp2,
                tc.tile_pool(name="sps", bufs=1, space="PSUM") as s_pool,
            ):
                hT = pp2.tile([128, BL, H, N], BF16)
                # persistent P^T buffers, one per head slot; pipeline sync
                # comes from emission order + region-level WAR/RAW deps
                pt_bufs = [pp2.tile([128, MC, N], BF16, name=f"ptb{i}")
                           for i in range(2)]
                # persistent round-robin es buffers (4 per head slot) with a
                # zeroed transpose pad
                es_bufs = [pp2.tile([128, 896], BF16, name=f"esb{j}")
                           for j in range(8)]
                for t in es_bufs:
                    nc.vector.memset(t[:, 784:], 0.0)
                es_cnt = [0, 0]
                s_rot = [0]
                hsl = pp2.tile([128, 8, 128], BF16, name="hsl")
                nc.vector.memset(hsl[:], 0.0)

                def softmax_tile(hp, b4, nt, defer_tp=None):
                    nr = _rows(nt)
                    c0 = b4 * N + nt * 128
                    nsl = slice(nt * 128, nt * 128 + nr)
                    # 3-way rotating S tiles shared by both heads (global
                    # stream counter): the WAR partner of each S matmul is
                    # the exp 3 stream-slots back, so the S->exp ping-pong
                    # never stalls either chain, including at batch bounds
                    Ss = []
                    for i in range(2):
                        j = s_rot[0] % 3
                        s_rot[0] += 1
                        Ss.append(s_pool.tile([128, 2, 512], F32,
                                              tag=f"sp{j}", name=f"sp{j}"))
                    for i in range(2):
                        h = 2 * hp + i
                        ccq, cck = (0, 1) if h < 4 else (2, 3)
                        pq = 32 * (h % 4)
                        for half in range(2):
                            m0 = b4 * N + half * 392
                            nc.tensor.matmul(
                                Ss[i][:nr, half, :392],
                                qkT[pq:pq + 32, ccq, c0:c0 + nr],
                                qkT[pq:pq + 32, cck, m0:m0 + 392],
                                start=True, stop=True,
                                tile_position=(pq, 0))
                    for i in range(2):
                        es = es_bufs[4 * i + es_cnt[i] % 4]
                        es_cnt[i] += 1
                        den = ap_.tile([128, 1], F32, tag=f"den{i}")
                        nc.scalar.activation(
                            es[:nr, :784].rearrange("p (a b) -> p a b", a=2),
                            Ss[i][:nr, :, :392], AF.Exp)
                        nc.gpsimd.tensor_tensor(
                            es[:nr, :784], es[:nr, :784],
                            ebt[i][:nr, nt, :], OP.mult)
                        nc.vector.tensor_scalar(
                            es[:nr, :784], es[:nr, :784], 1.0, 0.0,
                            OP.mult, OP.add, accum_out=den[:nr])
                        rd = ap_.tile([128, 1], F32, tag=f"rd{i}")
                        nc.vector.reciprocal(rd[:nr], den[:nr])
                        nc.vector.tensor_scalar_mul(
                            es[:nr, :784], es[:nr, :784], rd[:nr, 0:1])
                        tp = partial(nc.sync.dma_start_transpose,
                                     pt_bufs[i][:, :, nsl], es[:nr, :896])
                        if defer_tp is None:
                            tp()
                        else:
                            defer_tp.append(tp)

                def av_group(hp, b4, k):
                    # one (head, col-half) AV + hardswish group of batch b4
                    i = k // 2
                    hs, hw_ = ((0, 448), (448, 336))[k % 2]
                    h = 2 * hp + i
                    ops = mm_pool.tile([128, 512], F32, tag="mm")
                    for mc in range(MC):
                        mr = _rows(mc)
                        nc.tensor.matmul(
                            ops[:, :hw_],
                            v_sb[:mr, b4, mc, h * 128:(h + 1) * 128],
                            pt_bufs[i][:mr, mc, hs:hs + hw_],
                            start=(mc == 0), stop=(mc == 6))
                    tt = ap_.tile([128, 448], F32, tag="hsw")
                    nc.vector.tensor_scalar(
                        tt[:, :hw_], ops[:, :hw_],
                        bv3_sb[:, h:h + 1], 0.0, OP.add, OP.max)
                    mm_ = ap_.tile([128, 448], F32, tag="mmin")
                    nc.gpsimd.tensor_scalar_min(
                        mm_[:, :hw_], tt[:, :hw_], 6.0)
                    nc.vector.scalar_tensor_tensor(
                        hT[:, b4, h, hs:hs + hw_],
                        tt[:, :hw_], -3.0, mm_[:, :hw_],
                        OP.add, OP.mult)

                def proj_group(b4, nt):
                    c0 = b4 * N + nt * 128
                    ps = mm_pool.tile([128, 512], F32, tag="mm")
                    for hh in range(8):
                        nc.tensor.matmul(
                            ps[:],
                            hT[:, b4, hh, nt * 128:nt * 128 + 128],
                            wp_sb[:, hh, :],
                            start=(hh == 0), stop=(hh == 7))
                    ob = ap_.tile([128, 512], BF16, tag="ob")
                    nc.vector.tensor_tensor(ob[:], ps[:], bp_sb[:], OP.add)
                    nc.sync.dma_start(out[c0:c0 + 128, :], ob[:])

                def hsl_copy(hh):
                    nc.vector.tensor_copy(
                        hsl[:, hh].rearrange(
                            "p (b j) -> p b j", b=BL)[:, :, :16],
                        hT[:, :, hh, 768:784])

                def proj_sliver():
                    # 16-row tail tokens of all 4 batches, 32-aligned packing
                    # (heads 0-5 staged early; only the last pair here)
                    for hh in (6, 7):
                        hsl_copy(hh)
                    ps = mm_pool.tile([128, 512], F32, tag="mm")
                    for hh in range(8):
                        nc.tensor.matmul(
                            ps[:], hsl[:, hh], wp_sb[:, hh, :],
                            start=(hh == 0), stop=(hh == 7))
                    obs = ap_.tile([128, 512], BF16, tag="ob")
                    nc.vector.tensor_tensor(obs[:], ps[:], bp_sb[:], OP.add)
                    for b4 in range(BL):
                        nc.sync.dma_start(
                            out[b4 * N + 768:b4 * N + 784, :],
                            obs[32 * b4:32 * b4 + 16])

                # schedule: slot s of batch b4 runs softmax_tile(nt=s) then
                # filler groups, placed >=2 slots after the softmax chain
                # they consume: hs=448 AV groups of the PREVIOUS batch in
                # slots 2-3 (chain of its nt6 ends ~2 slots into this batch;
                # pt cols 448:784 are only overwritten from nt=3 on, emitted
                # after); hs=0 AV groups of the CURRENT batch in slots 5-6
                # (their pt cols 0:448 complete at nt=3); projections of the
                # previous batch follow its hs=448 hardswish in slots 4-6.
                prev = None
                for hp in range(4):
                    for b4 in range(BL):
                        slotfill = [[] for _ in range(NT)]
                        if prev is not None:
                            php, pb4 = prev
                            # both hs=448 groups must precede this batch's
                            # nt3 transpose (it overwrites pt cols 448:512);
                            # that transpose is deferred to slot 4, so slot 3
                            # is the last legal (and least stalled) position
                            slotfill[3].append(partial(av_group, php, pb4, 1))
                            slotfill[3].append(partial(av_group, php, pb4, 3))
                            if php == 3:
                                pj = [partial(proj_group, pb4, n_)
                                      for n_ in range(6)]
                                slotfill[4] += pj[0:2]
                                slotfill[5] += pj[2:4]
                                slotfill[6] += pj[4:6]
                            if pb4 == 3:
                                # stage the finished era's hT tail-token
                                # slivers (writers ran in slot 3)
                                slotfill[5].append(
                                    partial(hsl_copy, 2 * php))
                                slotfill[5].append(
                                    partial(hsl_copy, 2 * php + 1))
                        slotfill[5].append(partial(av_group, hp, b4, 0))
                        slotfill[6].append(partial(av_group, hp, b4, 2))
                        for nt in range(NT):
                            # bias-table chunk loads (hp0's full table rode
                            # the stage-1 stream): chunks 0-3 prefetched at
                            # the previous batch 3, chunks 4-6 early in the
                            # era's own batch 0, queued behind the
                            # boundary-critical transposes
                            if b4 == 0 and hp > 0 and 1 <= nt < 4:
                                for i in range(2):
                                    eb_chunk(hp, i, nt + 3)
                            if nt == 3:
                                # run the chain but hold nt3's transposes
                                # until after slot 3's hs=448 AV fillers
                                deferred = []
                                softmax_tile(hp, b4, nt, defer_tp=deferred)
                            else:
                                if nt == 4:
                                    for tp in deferred:
                                        tp()
                                softmax_tile(hp, b4, nt)
                            if b4 == BL - 1 and hp < 3 and nt >= 3:
                                # queue behind this batch's late transposes
                                # (the era-boundary AV fillers wait on them)
                                for i in range(2):
                                    eb_chunk(hp + 1, i, nt - 3)
                            for f in slotfill[nt]:
                                f()
                        prev = (hp, b4)
                # drain: last batch's hs=448 AV groups + projection; the
                # sliver projection goes first so its small out-DMAs overlap
                # the main projection matmuls instead of trailing them
                av_group(3, 3, 1)
                av_group(3, 3, 3)
                proj_sliver()
                for n_ in range(6):
                    proj_group(3, n_)

    nc.compile()
    return nc


_NC = None


def _prep_weights(qkv_w, qkv_g, qkv_b, qkv_m, qkv_v, ab, proj_w, proj_g,
                  proj_b, proj_m, proj_v, idxs):
    s = qkv_g / np.sqrt(qkv_v + EPS)
    W = qkv_w * s[:, None]                       # [1536, 512]
    bias = qkv_b - qkv_m * s                     # [1536]
    scale = KD ** -0.5
    # head-grouped reorder: chunk0=q0..3, chunk1=k0..3, chunk2=q4..7, chunk3=k4..7
    qk_rows, v_rows = [], []
    for h in range(H):
        base = h * (2 * KD + VD)
        qk_rows.append((np.arange(base, base + KD), True))
        qk_rows.append((np.arange(base + KD, base + 2 * KD), False))
        v_rows.append(np.arange(base + 2 * KD, base + 2 * KD + VD))
    order = []
    for grp in range(4):
        half = grp // 2
        is_q = (grp % 2 == 0)
        for hh in range(4 * half, 4 * half + 4):
            order.append((qk_rows[2 * hh][0] if is_q else qk_rows[2 * hh + 1][0], is_q))
    wqk = np.empty((512, 512), np.float32)
    bqk = np.empty(512, np.float32)
    o = 0
    for rows, is_q in order:
        f = scale if is_q else 1.0
        wqk[:, o:o + KD] = (W[rows] * f).T
        bqk[o:o + KD] = bias[rows] * f
        o += KD
    vr = np.concatenate(v_rows)
    wv = W[vr].T.copy()                          # [512, 1024]
    bv = bias[vr]                                # folded into hardswish bias

    sp = proj_g / np.sqrt(proj_v + EPS)
    # reference: out = h @ proj_w.T * sp + (proj_b - proj_m*sp); fold 1/6 of hswish
    wp = (proj_w * sp[:, None]).T.astype(np.float32) / 6.0   # [1024, 512]
    bp = proj_b - proj_m * sp

    btab = ab[:, idxs].astype(np.float32)                    # [H, 784, 784]
    eb_pad = np.zeros((H, 896, 784), np.float32)
    eb_pad[:, :784] = np.exp(btab)               # multiplicative bias

    return dict(
        wqk=wqk.astype(ml_dtypes.bfloat16),
        bqk=bqk.reshape(4, 128).T.copy(),
        wv=wv.astype(ml_dtypes.bfloat16),
        bv3=(3.0 + bv).reshape(8, 128).T.astype(np.float32).copy(),
        wp=wp.astype(ml_dtypes.bfloat16),
        bp=np.broadcast_to(bp, (128, 512)).astype(np.float32).copy(),
        ebias=eb_pad.astype(ml_dtypes.bfloat16),
    )


def kernel(x, qkv_w, qkv_g, qkv_b, qkv_m, qkv_v, ab,
           proj_w, proj_g, proj_b, proj_m, proj_v, idxs, _trace=False):
    global _NC
    x = np.asarray(x, np.float32)
    shared = _prep_weights(
        np.asarray(qkv_w, np.float32), np.asarray(qkv_g, np.float32),
        np.asarray(qkv_b, np.float32), np.asarray(qkv_m, np.float32),
        np.asarray(qkv_v, np.float32), np.asarray(ab, np.float32),
        np.asarray(proj_w, np.float32), np.asarray(proj_g, np.float32),
        np.asarray(proj_b, np.float32), np.asarray(proj_m, np.float32),
        np.asarray(proj_v, np.float32), np.asarray(idxs))

    if _NC is None:
        _NC = build_nc()
    nc = _NC

    in_maps = []
    for c in range(NCORES):
        xs = x[c * BL:(c + 1) * BL]                      # [4, 784, 512]
        xT = xs.transpose(2, 0, 1).reshape(DIM, NL).astype(ml_dtypes.bfloat16)
        m = dict(shared)
        m["xT"] = xT
        in_maps.append(m)

    res = run_bass_kernel_spmd(nc, in_maps, core_ids=list(range(NCORES)),
                               trace=_trace)
    outs = [res.results[c]["out"].astype(np.float32).reshape(BL, N, DIM)
            for c in range(NCORES)]
    full = np.concatenate(outs, axis=0)
    if _trace:
        return full, res.exec_time_ns
    return full
